# revision 7
# baseline (speedup 1.0000x reference)
"""
Causal self-attention (single head) on 8 trn2 NeuronCores.

Problem: x[4, 2048, 1024], Wq/Wk/Wv[1024, 1024] (torch Linear layout [d_out, d_in]).
    q/k/v = x @ W.T ; out = softmax(mask(q k^T) / 32) @ v

Sharding (no collectives, uniform SPMD program):
  core c -> batch b = c // 2, role r = c % 2.
  Both cores of a pair compute K/V projections for the full 2048-row
  sequence of their batch (duplicated work, avoids cross-core comms).
  Query rows are split between the pair in 4 i-blocks of 256 rows with
  per-slot padded causal extents JT_SLOTS = [4, 8, 12, 16] identical for
  both roles; causality inside the padded slots is enforced with a
  per-core "delta" input (keep iff jj - ii <= delta).

Precision plan (error gate is 2e-2 relative; measured headroom ~10x):
  fp16 is the working dtype (all |values| < 6e4, 10-bit mantissa).
  K-projection runs as fp8e4m3 DoubleRow matmuls (2 weights/PE cell,
  2x MACs/cycle); q/k are stored fp16 and scores run fp16 (storing them
  fp8 + fp8 scores measured 2.4e-2 rel err -- over the gate; fp8 only in
  the K projection measures under it). V path stays fp16 throughout.

Everything is SBUF-resident (~170 KB of the 208 KB/partition): x in
fp16 + fp8, all three weights, kT8/qT8, and the 16 v tiles. No DRAM
spills, so the only DMA is inputs in (~11 MB) and the output out (4 MB).

Phase B is software-pipelined: scores(t+1) is issued to the PE before
ctx(t), so the activation-engine exp(t) latency hides behind the next
scores matmul and the PE never idles (HAM clock gate stays at 8/8).
"""

import sys

for _p in ("/opt/trn_rl_repo", "/root/.axon_site/_ro/trn_rl_repo"):
    if _p not in sys.path:
        sys.path.append(_p)

import numpy as np
import ml_dtypes

import concourse.bass as bass
import concourse.mybir as mybir
import concourse.tile as tile
from concourse import bacc
from concourse.bass_utils import run_bass_kernel_spmd
import concourse.bass_utils as _bu

# NOTE: walrus --enable-ldw-opt stays at its default (false): the measured
# back-to-back MM rate is full speed without it for fp16/fp8 (216.5 ns per
# 512-row MM), and enabling it rejects DoubleRow LDWEIGHTS instructions.

F32 = mybir.dt.float32
F16 = mybir.dt.float16
F8 = mybir.dt.float8e4
DR = mybir.MatmulPerfMode.DoubleRow

B, S, D = 4, 2048, 1024
P = 128
ND = D // P          # 8 d-tiles (projection contraction)
NO = D // P          # 8 o-tiles
IB = 256             # i-block (query block) rows
N_IB = 4
JT_SLOTS = [4, 8, 12, 16]
ROLE_STARTS = {
    0: [0, 768, 1024, 1792],
    1: [256, 512, 1280, 1536],
}
N_CORES = 8
N_WARM = 18


def _mm(nc, out, lhsT, rhs, start, stop, perf_mode=None):
    nc.tensor.matmul(out, lhsT, rhs, start=start, stop=stop,
                     perf_mode=perf_mode)


def build_program():
    nc = bacc.Bacc(
        "TRN2",
        target_bir_lowering=False,
        debug=False,
        enable_asserts=False,
        num_devices=N_CORES,
    )
    xT16_in = nc.dram_tensor("xT16", [D, S], F16, kind="ExternalInput").ap()
    x8_in = nc.dram_tensor("x8", [D, S], F8, kind="ExternalInput").ap()
    xqT_in = nc.dram_tensor("xqT", [D, N_IB * IB], F16, kind="ExternalInput").ap()
    wq_in = nc.dram_tensor("wqT", [D, D], F16, kind="ExternalInput").ap()
    wk_in = nc.dram_tensor("wkT", [D, D], F8, kind="ExternalInput").ap()
    wv_in = nc.dram_tensor("wvT", [D, D], F16, kind="ExternalInput").ap()
    t0_in = nc.dram_tensor("t0", [P, IB], F16, kind="ExternalInput").ap()
    delta_in = nc.dram_tensor("delta", [P, 16], F16, kind="ExternalInput").ap()
    ones_in = nc.dram_tensor("ones", [P, 2], F16, kind="ExternalInput").ap()
    out = nc.dram_tensor("out", [N_IB * IB, D], F32, kind="ExternalOutput").ap()

    scale = 1.0 / 32.0  # 1/sqrt(d_v)

    def d_major(ap2d):
        # [ND*P, C] DRAM view -> [P, ND, C] (partition-major 3D AP)
        return ap2d.rearrange("(nd p) c -> p nd c", p=P)

    with tile.TileContext(nc) as tc:
        with tc.tile_pool(name="res", bufs=1) as rp:
            # ---- constants first (warm-up stationary/moving data) ----
            t0_t = rp.tile([P, IB], F16, tag="t0")
            nc.gpsimd.dma_start(t0_t[:], t0_in[:])
            delta_t = rp.tile([P, 16], F16, tag="delta")
            nc.gpsimd.dma_start(delta_t[:], delta_in[:])
            ones_t = rp.tile([P, 2], F16, tag="ones")
            nc.gpsimd.dma_start(ones_t[:], ones_in[:])

            # ---- resident tensors + their loads ----
            xq16 = rp.tile([P, ND, N_IB * IB], F16, tag="xq16")
            # sb chunks so Q-proj's first chunk can start early
            nc.scalar.dma_start(xq16[:, :, 0:512], d_major(xqT_in[:, 0:512]))
            nc.scalar.dma_start(xq16[:, :, 512:1024], d_major(xqT_in[:, 512:1024]))
            wq16 = rp.tile([P, ND, D], F16, tag="wq16")
            nc.sync.dma_start(wq16[:, :, 0:512], d_major(wq_in[:, 0:512]))
            nc.sync.dma_start(wq16[:, :, 512:1024], d_major(wq_in[:, 512:1024]))
            x8t = rp.tile([P, ND, S], F8, tag="x8t")
            nc.gpsimd.dma_start(x8t[:, :, 0:1024], d_major(x8_in[:, 0:1024]))
            nc.gpsimd.dma_start(x8t[:, :, 1024:2048], d_major(x8_in[:, 1024:2048]))
            wk8 = rp.tile([P, ND, D], F8, tag="wk8")
            nc.sync.dma_start(wk8[:], d_major(wk_in))
            xt16 = rp.tile([P, ND, S], F16, tag="xt16")
            for c4 in range(4):
                nc.scalar.dma_start(
                    xt16[:, :, c4 * 512:(c4 + 1) * 512],
                    d_major(xT16_in[:, c4 * 512:(c4 + 1) * 512]),
                )
            wv16 = rp.tile([P, ND, D], F16, tag="wv16")
            nc.sync.dma_start(wv16[:], d_major(wv_in))

            kT16 = rp.tile([P, NO, S], F16, tag="kT16")
            qT16 = rp.tile([P, NO, N_IB * IB], F16, tag="qT16")
            v_tiles = [
                rp.tile([P, D], F16, tag=f"v{j}", name=f"v{j}")
                for j in range(S // P)
            ]

            # ---- PE warm-up on an on-chip memset tile: no DMA dependency,
            # so the HAM ramp starts right after the framework preamble and
            # covers the ~7us DMA cold-start of the first real loads.
            warm16 = rp.tile([P, IB], F16, tag="warm16")
            nc.vector.memset(warm16[:], 1.0)
            psA_cm = tc.tile_pool(name="psA", bufs=2, space="PSUM")
            psA = psA_cm.__enter__()
            wps = psA.tile([P, IB], F32, tag="wps", name="wps", bufs=1)
            for w in range(N_WARM):
                _mm(nc, wps[:], warm16[:, 0:P], warm16[:], start=True, stop=True)

            # ---------------- Phase A: projections ----------------
            # Q: fp16, psum [o 128, i 512]; store as fp8 for the scores mm
            for sb in range(2):
                for o in range(NO):
                    pq = psA.tile([P, 512], F32, tag="pp", name=f"pq{sb}_{o}")
                    for d in range(ND):
                        _mm(nc, pq[:],
                            wq16[:, d, o * P:(o + 1) * P],
                            xq16[:, d, sb * 512:(sb + 1) * 512],
                            start=(d == 0), stop=(d == ND - 1))
                    nc.vector.tensor_copy(
                        qT16[:, o, sb * 512:(sb + 1) * 512], pq[:])

            # K: fp8 DoubleRow, psum [o 128, j 512]; store fp8
            for jb in range(S // 512):
                for o in range(NO):
                    pk = psA.tile([P, 512], F32, tag="pp", name=f"pk{jb}_{o}")
                    for g in range(ND // 2):
                        _mm(nc, pk[:],
                            wk8[:, 2 * g:2 * g + 2, o * P:(o + 1) * P],
                            x8t[:, 2 * g:2 * g + 2, jb * 512:(jb + 1) * 512],
                            start=(g == 0), stop=(g == ND // 2 - 1),
                            perf_mode=DR)
                    nc.vector.tensor_copy(
                        kT16[:, o, jb * 512:(jb + 1) * 512], pk[:])

            # V: fp16, psum [j 128, o 512]; x tile stationary, wv moving
            for jb in range(S // 512):
                for jj in range(4):
                    jt = jb * 4 + jj
                    for ob in range(2):
                        pv = psA.tile([P, 512], F32, tag="pp", name=f"pv{jt}_{ob}")
                        for d in range(ND):
                            _mm(nc, pv[:],
                                xt16[:, d, jt * P:(jt + 1) * P],
                                wv16[:, d, ob * 512:(ob + 1) * 512],
                                start=(d == 0), stop=(d == ND - 1))
                        nc.vector.tensor_copy(
                            v_tiles[jt][:, ob * 512:(ob + 1) * 512], pv[:])

            psA_cm.__exit__(None, None, None)

            # ---------------- Phase B: attention ----------------
            with (
                tc.tile_pool(name="ex", bufs=3) as expool,
                tc.tile_pool(name="ost", bufs=2) as ostpool,
                tc.tile_pool(name="rcp", bufs=4) as rcpool,
                tc.tile_pool(name="psS", bufs=2, space="PSUM") as psS,
                tc.tile_pool(name="psC", bufs=1, space="PSUM") as psC,
                tc.tile_pool(name="psD", bufs=1, space="PSUM") as psD,
            ):
                def emit_scores(s, t, ps):
                    for o in range(NO):
                        _mm(nc, ps[:],
                            kT16[:, o, t * P:(t + 1) * P],
                            qT16[:, o, s * IB:(s + 1) * IB],
                            start=(o == 0), stop=(o == NO - 1))

                for s in reversed(range(N_IB)):
                    jt_n = JT_SLOTS[s]
                    cps = [
                        [
                            psC.tile([P, 512], F32, tag=f"c{it}{ob}",
                                     name=f"c{s}_{it}{ob}")
                            for ob in range(2)
                        ]
                        for it in range(2)
                    ]
                    dps = [
                        psD.tile([P, 2], F32, tag=f"d{it}", name=f"d{s}_{it}")
                        for it in range(2)
                    ]
                    ps_t = [None] * jt_n
                    ps_t[0] = psS.tile([P, IB], F32, tag="ps", name=f"ps{s}_0")
                    emit_scores(s, 0, ps_t[0])
                    for t in range(jt_n):
                        if t + 1 < jt_n:
                            ps_t[t + 1] = psS.tile([P, IB], F32, tag="ps",
                                                   name=f"ps{s}_{t + 1}")
                            emit_scores(s, t + 1, ps_t[t + 1])
                        ps = ps_t[t]
                        et = expool.tile([P, IB], F16, tag="et", name=f"et{s}_{t}")
                        if t >= jt_n - 4:
                            eraw = expool.tile([P, IB], F16, tag="eraw",
                                               name=f"er{s}_{t}")
                            nc.scalar.activation(
                                eraw[:], ps[:],
                                mybir.ActivationFunctionType.Exp, scale=scale,
                            )
                            col = s * 4 + (t - (jt_n - 4))
                            nc.vector.scalar_tensor_tensor(
                                et[:], t0_t[:], delta_t[:, col:col + 1], eraw[:],
                                op0=mybir.AluOpType.is_le,
                                op1=mybir.AluOpType.mult,
                            )
                        else:
                            nc.scalar.activation(
                                et[:], ps[:],
                                mybir.ActivationFunctionType.Exp, scale=scale,
                            )
                        last = t == jt_n - 1
                        for it in range(2):
                            lhs = et[:, it * P:(it + 1) * P]
                            for ob in range(2):
                                _mm(nc, cps[it][ob][:], lhs,
                                    v_tiles[t][:, ob * 512:(ob + 1) * 512],
                                    start=(t == 0), stop=last)
                            _mm(nc, dps[it][:], lhs, ones_t[:],
                                start=(t == 0), stop=last)
                    for it in range(2):
                        rc = rcpool.tile([P, 1], F32, tag="rc", name=f"rc{s}_{it}")
                        nc.vector.reciprocal(rc[:], dps[it][:, 0:1])
                        ot = ostpool.tile([P, D], F32, tag="ot", name=f"ot{s}_{it}")
                        rows = slice(s * IB + it * P, s * IB + (it + 1) * P)
                        for ob in range(2):
                            cols = slice(ob * 512, (ob + 1) * 512)
                            nc.vector.tensor_scalar_mul(
                                ot[:, cols], cps[it][ob][:], rc[:]
                            )
                            nc.sync.dma_start(out[rows, cols], ot[:, cols])

    nc.compile()
    return nc


_NC_CACHE = None


def _get_nc():
    global _NC_CACHE
    if _NC_CACHE is None:
        _NC_CACHE = build_program()
    return _NC_CACHE


def make_core_inputs(x, Wq, Wk, Wv):
    """Host-side shard prep. Returns list of 8 in_maps."""
    x = np.asarray(x, dtype=np.float32)
    wqT = np.ascontiguousarray(np.asarray(Wq, np.float32).T.astype(np.float16))
    wkT = np.ascontiguousarray(
        np.asarray(Wk, np.float32).T.astype(ml_dtypes.float8_e4m3))
    wvT = np.ascontiguousarray(np.asarray(Wv, np.float32).T.astype(np.float16))
    t0 = (np.arange(P, dtype=np.float32)[:, None]
          - np.arange(IB, dtype=np.float32)[None, :]).astype(np.float16)
    t0 = np.ascontiguousarray(t0)

    in_maps = []
    for c in range(N_CORES):
        b, r = divmod(c, 2)
        starts = ROLE_STARTS[r]
        xT = np.ascontiguousarray(x[b].T)
        xq = np.concatenate([x[b][i0:i0 + IB, :] for i0 in starts], axis=0)
        xqT = np.ascontiguousarray(xq.T.astype(np.float16))
        delta = np.empty((P, 16), np.float16)
        for s in range(N_IB):
            for tr in range(4):
                t = JT_SLOTS[s] - 4 + tr
                delta[:, s * 4 + tr] = float(starts[s] - P * t)
        in_maps.append({
            "xT16": xT.astype(np.float16),
            "x8": xT.astype(ml_dtypes.float8_e4m3),
            "xqT": xqT,
            "wqT": wqT, "wkT": wkT, "wvT": wvT,
            "t0": t0, "delta": np.ascontiguousarray(delta),
            "ones": np.ones((P, 2), np.float16),
        })
    return in_maps


def assemble_output(results):
    """Gather 8 per-core [1024, 1024] outputs into [B, S, D]."""
    out = np.empty((B, S, D), np.float32)
    for c in range(N_CORES):
        b, r = divmod(c, 2)
        starts = ROLE_STARTS[r]
        oc = results[c]["out"]
        for s, i0 in enumerate(starts):
            out[b, i0:i0 + IB, :] = oc[s * IB:(s + 1) * IB, :]
    return out


def kernel(x, Wq, Wk, Wv):
    nc = _get_nc()
    in_maps = make_core_inputs(x, Wq, Wk, Wv)
    res = run_bass_kernel_spmd(nc, in_maps, list(range(N_CORES)))
    return assemble_output(res.results)


# revision 8
# speedup vs baseline: 1.0078x; 1.0078x over previous
"""
Causal self-attention (single head) on 8 trn2 NeuronCores.

Problem: x[4, 2048, 1024], Wq/Wk/Wv[1024, 1024] (torch Linear layout [d_out, d_in]).
    q/k/v = x @ W.T ; out = softmax(mask(q k^T) / 32) @ v

Sharding (no collectives, uniform SPMD program):
  core c -> batch b = c // 2, role r = c % 2.
  Both cores of a pair compute K/V projections for the full 2048-row
  sequence of their batch (duplicated work, avoids cross-core comms).
  Query rows are split between the pair in 4 i-blocks of 256 rows with
  per-slot padded causal extents JT_SLOTS = [4, 8, 12, 16] identical for
  both roles; causality inside the padded slots is enforced with a
  per-core "delta" input (keep iff jj - ii <= delta).

Precision plan (error gate is 2e-2 relative; measured headroom ~10x):
  fp16 is the working dtype (all |values| < 6e4, 10-bit mantissa).
  K-projection runs as fp8e4m3 DoubleRow matmuls (2 weights/PE cell,
  2x MACs/cycle); q/k are stored fp16 and scores run fp16 (storing them
  fp8 + fp8 scores measured 2.4e-2 rel err -- over the gate; fp8 only in
  the K projection measures under it). V path stays fp16 throughout.

Everything is SBUF-resident (~170 KB of the 208 KB/partition): x in
fp16 + fp8, all three weights, kT8/qT8, and the 16 v tiles. No DRAM
spills, so the only DMA is inputs in (~11 MB) and the output out (4 MB).

Phase B is software-pipelined: scores(t+1) is issued to the PE before
ctx(t), so the activation-engine exp(t) latency hides behind the next
scores matmul and the PE never idles (HAM clock gate stays at 8/8).
"""

import sys

for _p in ("/opt/trn_rl_repo", "/root/.axon_site/_ro/trn_rl_repo"):
    if _p not in sys.path:
        sys.path.append(_p)

import numpy as np
import ml_dtypes

import concourse.bass as bass
import concourse.mybir as mybir
import concourse.tile as tile
from concourse import bacc
from concourse.bass_utils import run_bass_kernel_spmd
import concourse.bass_utils as _bu

# NOTE: walrus --enable-ldw-opt stays at its default (false): the measured
# back-to-back MM rate is full speed without it for fp16/fp8 (216.5 ns per
# 512-row MM), and enabling it rejects DoubleRow LDWEIGHTS instructions.

F32 = mybir.dt.float32
F16 = mybir.dt.float16
F8 = mybir.dt.float8e4
DR = mybir.MatmulPerfMode.DoubleRow

B, S, D = 4, 2048, 1024
P = 128
ND = D // P          # 8 d-tiles (projection contraction)
NO = D // P          # 8 o-tiles
IB = 256             # i-block (query block) rows
N_IB = 4
JT_SLOTS = [4, 8, 12, 16]
ROLE_STARTS = {
    0: [0, 768, 1024, 1792],
    1: [256, 512, 1280, 1536],
}
N_CORES = 8
N_WARM = 18


def _mm(nc, out, lhsT, rhs, start, stop, perf_mode=None):
    nc.tensor.matmul(out, lhsT, rhs, start=start, stop=stop,
                     perf_mode=perf_mode)


def build_program():
    nc = bacc.Bacc(
        "TRN2",
        target_bir_lowering=False,
        debug=False,
        enable_asserts=False,
        num_devices=N_CORES,
    )
    # Inputs are host-packed to the SBUF partition-major layout
    # [P, ND*C]: each DMA is 128 contiguous 16KB-ish runs instead of 1024
    # 1KB runs -- descriptor generation (~11ns each) was costing 5-11us of
    # engine issue time per load the d-major way.
    xT16_in = nc.dram_tensor("xT16", [P, ND * S], F16, kind="ExternalInput").ap()
    x8_in = nc.dram_tensor("x8", [P, ND * S], F8, kind="ExternalInput").ap()
    xqT_in = nc.dram_tensor("xqT", [P, ND * N_IB * IB], F16, kind="ExternalInput").ap()
    wq_in = nc.dram_tensor("wqT", [P, ND * D], F16, kind="ExternalInput").ap()
    wk_in = nc.dram_tensor("wkT", [P, ND * D], F8, kind="ExternalInput").ap()
    wv_in = nc.dram_tensor("wvT", [P, ND * D], F16, kind="ExternalInput").ap()
    t0_in = nc.dram_tensor("t0", [P, IB], F16, kind="ExternalInput").ap()
    delta_in = nc.dram_tensor("delta", [P, 16], F16, kind="ExternalInput").ap()
    ones_in = nc.dram_tensor("ones", [P, 2], F16, kind="ExternalInput").ap()
    out = nc.dram_tensor("out", [N_IB * IB, D], F32, kind="ExternalOutput").ap()

    scale = 1.0 / 32.0  # 1/sqrt(d_v)

    def packed(ap2d, c):
        # [P, ND*C] host-packed DRAM view -> [P, ND, C] 3D AP
        return ap2d.rearrange("p (nd c) -> p nd c", c=c)

    with tile.TileContext(nc) as tc:
        with tc.tile_pool(name="res", bufs=1) as rp:
            # ---- constants first (warm-up stationary/moving data) ----
            t0_t = rp.tile([P, IB], F16, tag="t0")
            nc.gpsimd.dma_start(t0_t[:], t0_in[:])
            delta_t = rp.tile([P, 16], F16, tag="delta")
            nc.gpsimd.dma_start(delta_t[:], delta_in[:])
            ones_t = rp.tile([P, 2], F16, tag="ones")
            nc.gpsimd.dma_start(ones_t[:], ones_in[:])

            # ---- resident tensors + their loads ----
            xq16 = rp.tile([P, ND, N_IB * IB], F16, tag="xq16")
            nc.scalar.dma_start(xq16[:], packed(xqT_in, N_IB * IB))
            wq16 = rp.tile([P, ND, D], F16, tag="wq16")
            nc.sync.dma_start(wq16[:], packed(wq_in, D))
            x8t = rp.tile([P, ND, S], F8, tag="x8t")
            nc.gpsimd.dma_start(x8t[:], packed(x8_in, S))
            wk8 = rp.tile([P, ND, D], F8, tag="wk8")
            nc.sync.dma_start(wk8[:], packed(wk_in, D))
            xt16 = rp.tile([P, ND, S], F16, tag="xt16")
            nc.scalar.dma_start(xt16[:], packed(xT16_in, S))
            wv16 = rp.tile([P, ND, D], F16, tag="wv16")
            nc.sync.dma_start(wv16[:], packed(wv_in, D))

            kT16 = rp.tile([P, NO, S], F16, tag="kT16")
            qT16 = rp.tile([P, NO, N_IB * IB], F16, tag="qT16")
            v_tiles = [
                rp.tile([P, D], F16, tag=f"v{j}", name=f"v{j}")
                for j in range(S // P)
            ]

            # ---- PE warm-up on an on-chip memset tile: no DMA dependency,
            # so the HAM ramp starts right after the framework preamble and
            # covers the ~7us DMA cold-start of the first real loads.
            warm16 = rp.tile([P, IB], F16, tag="warm16")
            nc.vector.memset(warm16[:], 1.0)
            psA_cm = tc.tile_pool(name="psA", bufs=2, space="PSUM")
            psA = psA_cm.__enter__()
            wps = psA.tile([P, IB], F32, tag="wps", name="wps", bufs=1)
            for w in range(N_WARM):
                _mm(nc, wps[:], warm16[:, 0:P], warm16[:], start=True, stop=True)

            # ---------------- Phase A: projections ----------------
            # Q: fp16, psum [o 128, i 512]; store as fp8 for the scores mm
            for sb in range(2):
                for o in range(NO):
                    pq = psA.tile([P, 512], F32, tag="pp", name=f"pq{sb}_{o}")
                    for d in range(ND):
                        _mm(nc, pq[:],
                            wq16[:, d, o * P:(o + 1) * P],
                            xq16[:, d, sb * 512:(sb + 1) * 512],
                            start=(d == 0), stop=(d == ND - 1))
                    nc.vector.tensor_copy(
                        qT16[:, o, sb * 512:(sb + 1) * 512], pq[:])

            # K: fp8 DoubleRow, psum [o 128, j 512]; store fp8
            for jb in range(S // 512):
                for o in range(NO):
                    pk = psA.tile([P, 512], F32, tag="pp", name=f"pk{jb}_{o}")
                    for g in range(ND // 2):
                        _mm(nc, pk[:],
                            wk8[:, 2 * g:2 * g + 2, o * P:(o + 1) * P],
                            x8t[:, 2 * g:2 * g + 2, jb * 512:(jb + 1) * 512],
                            start=(g == 0), stop=(g == ND // 2 - 1),
                            perf_mode=DR)
                    nc.vector.tensor_copy(
                        kT16[:, o, jb * 512:(jb + 1) * 512], pk[:])

            # V: fp16, psum [j 128, o 512]; x tile stationary, wv moving
            for jb in range(S // 512):
                for jj in range(4):
                    jt = jb * 4 + jj
                    for ob in range(2):
                        pv = psA.tile([P, 512], F32, tag="pp", name=f"pv{jt}_{ob}")
                        for d in range(ND):
                            _mm(nc, pv[:],
                                xt16[:, d, jt * P:(jt + 1) * P],
                                wv16[:, d, ob * 512:(ob + 1) * 512],
                                start=(d == 0), stop=(d == ND - 1))
                        nc.vector.tensor_copy(
                            v_tiles[jt][:, ob * 512:(ob + 1) * 512], pv[:])

            psA_cm.__exit__(None, None, None)

            # ---------------- Phase B: attention ----------------
            with (
                tc.tile_pool(name="ex", bufs=3) as expool,
                tc.tile_pool(name="ost", bufs=2) as ostpool,
                tc.tile_pool(name="rcp", bufs=4) as rcpool,
                tc.tile_pool(name="psS", bufs=2, space="PSUM") as psS,
                tc.tile_pool(name="psC", bufs=1, space="PSUM") as psC,
                tc.tile_pool(name="psD", bufs=1, space="PSUM") as psD,
            ):
                def emit_scores(s, t, ps):
                    for o in range(NO):
                        _mm(nc, ps[:],
                            kT16[:, o, t * P:(t + 1) * P],
                            qT16[:, o, s * IB:(s + 1) * IB],
                            start=(o == 0), stop=(o == NO - 1))

                for s in reversed(range(N_IB)):
                    jt_n = JT_SLOTS[s]
                    cps = [
                        [
                            psC.tile([P, 512], F32, tag=f"c{it}{ob}",
                                     name=f"c{s}_{it}{ob}")
                            for ob in range(2)
                        ]
                        for it in range(2)
                    ]
                    dps = [
                        psD.tile([P, 2], F32, tag=f"d{it}", name=f"d{s}_{it}")
                        for it in range(2)
                    ]
                    ps_t = [None] * jt_n
                    ps_t[0] = psS.tile([P, IB], F32, tag="ps", name=f"ps{s}_0")
                    emit_scores(s, 0, ps_t[0])
                    for t in range(jt_n):
                        if t + 1 < jt_n:
                            ps_t[t + 1] = psS.tile([P, IB], F32, tag="ps",
                                                   name=f"ps{s}_{t + 1}")
                            emit_scores(s, t + 1, ps_t[t + 1])
                        ps = ps_t[t]
                        et = expool.tile([P, IB], F16, tag="et", name=f"et{s}_{t}")
                        if t >= jt_n - 4:
                            eraw = expool.tile([P, IB], F16, tag="eraw",
                                               name=f"er{s}_{t}")
                            nc.scalar.activation(
                                eraw[:], ps[:],
                                mybir.ActivationFunctionType.Exp, scale=scale,
                            )
                            col = s * 4 + (t - (jt_n - 4))
                            nc.vector.scalar_tensor_tensor(
                                et[:], t0_t[:], delta_t[:, col:col + 1], eraw[:],
                                op0=mybir.AluOpType.is_le,
                                op1=mybir.AluOpType.mult,
                            )
                        else:
                            nc.scalar.activation(
                                et[:], ps[:],
                                mybir.ActivationFunctionType.Exp, scale=scale,
                            )
                        last = t == jt_n - 1
                        for it in range(2):
                            lhs = et[:, it * P:(it + 1) * P]
                            for ob in range(2):
                                _mm(nc, cps[it][ob][:], lhs,
                                    v_tiles[t][:, ob * 512:(ob + 1) * 512],
                                    start=(t == 0), stop=last)
                            _mm(nc, dps[it][:], lhs, ones_t[:],
                                start=(t == 0), stop=last)
                    for it in range(2):
                        rc = rcpool.tile([P, 1], F32, tag="rc", name=f"rc{s}_{it}")
                        nc.vector.reciprocal(rc[:], dps[it][:, 0:1])
                        ot = ostpool.tile([P, D], F32, tag="ot", name=f"ot{s}_{it}")
                        rows = slice(s * IB + it * P, s * IB + (it + 1) * P)
                        for ob in range(2):
                            cols = slice(ob * 512, (ob + 1) * 512)
                            nc.vector.tensor_scalar_mul(
                                ot[:, cols], cps[it][ob][:], rc[:]
                            )
                            nc.sync.dma_start(out[rows, cols], ot[:, cols])

    nc.compile()
    return nc


_NC_CACHE = None


def _get_nc():
    global _NC_CACHE
    if _NC_CACHE is None:
        _NC_CACHE = build_program()
    return _NC_CACHE


def _pack(a2d):
    """[ND*P, C] d-major -> [P, ND*C] partition-major (SBUF layout)."""
    d, c = a2d.shape
    return np.ascontiguousarray(
        a2d.reshape(ND, P, c).transpose(1, 0, 2).reshape(P, ND * c))


def make_core_inputs(x, Wq, Wk, Wv):
    """Host-side shard prep. Returns list of 8 in_maps."""
    x = np.asarray(x, dtype=np.float32)
    wqT = _pack(np.asarray(Wq, np.float32).T.astype(np.float16))
    wkT = _pack(np.asarray(Wk, np.float32).T.astype(ml_dtypes.float8_e4m3))
    wvT = _pack(np.asarray(Wv, np.float32).T.astype(np.float16))
    t0 = (np.arange(P, dtype=np.float32)[:, None]
          - np.arange(IB, dtype=np.float32)[None, :]).astype(np.float16)
    t0 = np.ascontiguousarray(t0)

    in_maps = []
    for c in range(N_CORES):
        b, r = divmod(c, 2)
        starts = ROLE_STARTS[r]
        xT = np.ascontiguousarray(x[b].T)
        xq = np.concatenate([x[b][i0:i0 + IB, :] for i0 in starts], axis=0)
        xqT = _pack(xq.T.astype(np.float16))
        delta = np.empty((P, 16), np.float16)
        for s in range(N_IB):
            for tr in range(4):
                t = JT_SLOTS[s] - 4 + tr
                delta[:, s * 4 + tr] = float(starts[s] - P * t)
        in_maps.append({
            "xT16": _pack(xT.astype(np.float16)),
            "x8": _pack(xT.astype(ml_dtypes.float8_e4m3)),
            "xqT": xqT,
            "wqT": wqT, "wkT": wkT, "wvT": wvT,
            "t0": t0, "delta": np.ascontiguousarray(delta),
            "ones": np.ones((P, 2), np.float16),
        })
    return in_maps


def assemble_output(results):
    """Gather 8 per-core [1024, 1024] outputs into [B, S, D]."""
    out = np.empty((B, S, D), np.float32)
    for c in range(N_CORES):
        b, r = divmod(c, 2)
        starts = ROLE_STARTS[r]
        oc = results[c]["out"]
        for s, i0 in enumerate(starts):
            out[b, i0:i0 + IB, :] = oc[s * IB:(s + 1) * IB, :]
    return out


def kernel(x, Wq, Wk, Wv):
    nc = _get_nc()
    in_maps = make_core_inputs(x, Wq, Wk, Wv)
    res = run_bass_kernel_spmd(nc, in_maps, list(range(N_CORES)))
    return assemble_output(res.results)


# revision 9
# speedup vs baseline: 1.0289x; 1.0210x over previous
"""
Causal self-attention (single head) on 8 trn2 NeuronCores.

Problem: x[4, 2048, 1024], Wq/Wk/Wv[1024, 1024] (torch Linear layout [d_out, d_in]).
    q/k/v = x @ W.T ; out = softmax(mask(q k^T) / 32) @ v

Sharding (no collectives, uniform SPMD program):
  core c -> batch b = c // 2, role r = c % 2.
  Both cores of a pair compute K/V projections for the full 2048-row
  sequence of their batch (duplicated work, avoids cross-core comms).
  Query rows are split between the pair in 4 i-blocks of 256 rows with
  per-slot padded causal extents JT_SLOTS = [4, 8, 12, 16] identical for
  both roles; causality inside the padded slots is enforced with a
  per-core "delta" input (keep iff jj - ii <= delta).

Precision plan (error gate is 2e-2 relative; measured headroom ~10x):
  fp16 is the working dtype (all |values| < 6e4, 10-bit mantissa).
  K-projection runs as fp8e4m3 DoubleRow matmuls (2 weights/PE cell,
  2x MACs/cycle); q/k are stored fp16 and scores run fp16 (storing them
  fp8 + fp8 scores measured 2.4e-2 rel err -- over the gate; fp8 only in
  the K projection measures under it). V path stays fp16 throughout.

Everything is SBUF-resident (~170 KB of the 208 KB/partition): x in
fp16 + fp8, all three weights, kT8/qT8, and the 16 v tiles. No DRAM
spills, so the only DMA is inputs in (~11 MB) and the output out (4 MB).

Phase B is software-pipelined: scores(t+1) is issued to the PE before
ctx(t), so the activation-engine exp(t) latency hides behind the next
scores matmul and the PE never idles (HAM clock gate stays at 8/8).
"""

import sys

for _p in ("/opt/trn_rl_repo", "/root/.axon_site/_ro/trn_rl_repo"):
    if _p not in sys.path:
        sys.path.append(_p)

import numpy as np
import ml_dtypes

import concourse.bass as bass
import concourse.mybir as mybir
import concourse.tile as tile
from concourse import bacc
from concourse.bass_utils import run_bass_kernel_spmd
import concourse.bass_utils as _bu

# NOTE: walrus --enable-ldw-opt stays at its default (false): the measured
# back-to-back MM rate is full speed without it for fp16/fp8 (216.5 ns per
# 512-row MM), and enabling it rejects DoubleRow LDWEIGHTS instructions.

F32 = mybir.dt.float32
F16 = mybir.dt.float16
F8 = mybir.dt.float8e4
DR = mybir.MatmulPerfMode.DoubleRow

B, S, D = 4, 2048, 1024
P = 128
ND = D // P          # 8 d-tiles (projection contraction)
NO = D // P          # 8 o-tiles
IB = 256             # i-block (query block) rows
N_IB = 4
JT_SLOTS = [4, 8, 12, 16]
ROLE_STARTS = {
    0: [0, 768, 1024, 1792],
    1: [256, 512, 1280, 1536],
}
N_CORES = 8
N_WARM = 22


def _mm(nc, out, lhsT, rhs, start, stop, perf_mode=None):
    nc.tensor.matmul(out, lhsT, rhs, start=start, stop=stop,
                     perf_mode=perf_mode)


def build_program():
    nc = bacc.Bacc(
        "TRN2",
        target_bir_lowering=False,
        debug=False,
        enable_asserts=False,
        num_devices=N_CORES,
    )
    # Inputs are host-packed to the SBUF partition-major layout
    # [P, ND*C]: each DMA is 128 contiguous 16KB-ish runs instead of 1024
    # 1KB runs -- descriptor generation (~11ns each) was costing 5-11us of
    # engine issue time per load the d-major way.
    xT16_in = nc.dram_tensor("xT16", [P, ND * S], F16, kind="ExternalInput").ap()
    x8_in = nc.dram_tensor("x8", [P, ND * S], F8, kind="ExternalInput").ap()
    xqT_in = nc.dram_tensor("xqT", [P, ND * N_IB * IB], F16, kind="ExternalInput").ap()
    wq_in = nc.dram_tensor("wqT", [P, ND * D], F16, kind="ExternalInput").ap()
    wk_in = nc.dram_tensor("wkT", [P, ND * D], F8, kind="ExternalInput").ap()
    wv_in = nc.dram_tensor("wvT", [P, ND * D], F16, kind="ExternalInput").ap()
    t0_in = nc.dram_tensor("t0", [P, IB], F16, kind="ExternalInput").ap()
    delta_in = nc.dram_tensor("delta", [P, 16], F16, kind="ExternalInput").ap()
    ones_in = nc.dram_tensor("ones", [P, 2], F16, kind="ExternalInput").ap()
    out = nc.dram_tensor("out", [N_IB * IB, D], F16, kind="ExternalOutput").ap()

    scale = 1.0 / 32.0  # 1/sqrt(d_v)

    def packed(ap2d, c):
        # [P, ND*C] host-packed DRAM view -> [P, ND, C] 3D AP
        return ap2d.rearrange("p (nd c) -> p nd c", c=c)

    with tile.TileContext(nc) as tc:
        with tc.tile_pool(name="res", bufs=1) as rp:
            # ---- constants first (warm-up stationary/moving data) ----
            t0_t = rp.tile([P, IB], F16, tag="t0")
            nc.gpsimd.dma_start(t0_t[:], t0_in[:])
            delta_t = rp.tile([P, 16], F16, tag="delta")
            nc.gpsimd.dma_start(delta_t[:], delta_in[:])
            ones_t = rp.tile([P, 2], F16, tag="ones")
            nc.gpsimd.dma_start(ones_t[:], ones_in[:])

            # ---- resident tensors + their loads ----
            # Two DMA queues, each FIFO-serialized in consumption order so
            # the Q-projection inputs get the full HBM bandwidth first
            # (concurrent queues split it and delayed Q's start by ~11us).
            xq16 = rp.tile([P, ND, N_IB * IB], F16, tag="xq16")
            nc.scalar.dma_start(xq16[:], packed(xqT_in, N_IB * IB))
            wq16 = rp.tile([P, ND, D], F16, tag="wq16")
            nc.sync.dma_start(wq16[:], packed(wq_in, D))
            wk8 = rp.tile([P, ND, D], F8, tag="wk8")
            nc.sync.dma_start(wk8[:], packed(wk_in, D))
            x8t = rp.tile([P, ND, S], F8, tag="x8t")
            nc.scalar.dma_start(x8t[:], packed(x8_in, S))
            wv16 = rp.tile([P, ND, D], F16, tag="wv16")
            nc.sync.dma_start(wv16[:], packed(wv_in, D))
            xt16 = rp.tile([P, ND, S], F16, tag="xt16")
            nc.scalar.dma_start(xt16[:], packed(xT16_in, S))

            kT16 = rp.tile([P, NO, S], F16, tag="kT16")
            qT16 = rp.tile([P, NO, N_IB * IB], F16, tag="qT16")
            v_tiles = [
                rp.tile([P, D], F16, tag=f"v{j}", name=f"v{j}")
                for j in range(S // P)
            ]

            # ---- PE warm-up on an on-chip memset tile: no DMA dependency,
            # so the HAM ramp starts right after the framework preamble and
            # covers the ~7us DMA cold-start of the first real loads.
            warm16 = rp.tile([P, IB], F16, tag="warm16")
            nc.vector.memset(warm16[:], 1.0)
            psA_cm = tc.tile_pool(name="psA", bufs=2, space="PSUM")
            psA = psA_cm.__enter__()
            wps = psA.tile([P, IB], F32, tag="wps", name="wps", bufs=1)
            for w in range(N_WARM):
                _mm(nc, wps[:], warm16[:, 0:P], warm16[:], start=True, stop=True)

            # ---------------- Phase A: projections ----------------
            # Q: fp16, psum [o 128, i 512]; store as fp8 for the scores mm
            for sb in range(2):
                for o in range(NO):
                    pq = psA.tile([P, 512], F32, tag="pp", name=f"pq{sb}_{o}")
                    for d in range(ND):
                        _mm(nc, pq[:],
                            wq16[:, d, o * P:(o + 1) * P],
                            xq16[:, d, sb * 512:(sb + 1) * 512],
                            start=(d == 0), stop=(d == ND - 1))
                    nc.vector.tensor_copy(
                        qT16[:, o, sb * 512:(sb + 1) * 512], pq[:])

            # K: fp8 DoubleRow, psum [o 128, j 512]; store fp8
            for jb in range(S // 512):
                for o in range(NO):
                    pk = psA.tile([P, 512], F32, tag="pp", name=f"pk{jb}_{o}")
                    for g in range(ND // 2):
                        _mm(nc, pk[:],
                            wk8[:, 2 * g:2 * g + 2, o * P:(o + 1) * P],
                            x8t[:, 2 * g:2 * g + 2, jb * 512:(jb + 1) * 512],
                            start=(g == 0), stop=(g == ND // 2 - 1),
                            perf_mode=DR)
                    nc.vector.tensor_copy(
                        kT16[:, o, jb * 512:(jb + 1) * 512], pk[:])

            # V: fp16, psum [j 128, o 512]; x tile stationary, wv moving
            for jb in range(S // 512):
                for jj in range(4):
                    jt = jb * 4 + jj
                    for ob in range(2):
                        pv = psA.tile([P, 512], F32, tag="pp", name=f"pv{jt}_{ob}")
                        for d in range(ND):
                            _mm(nc, pv[:],
                                xt16[:, d, jt * P:(jt + 1) * P],
                                wv16[:, d, ob * 512:(ob + 1) * 512],
                                start=(d == 0), stop=(d == ND - 1))
                        nc.vector.tensor_copy(
                            v_tiles[jt][:, ob * 512:(ob + 1) * 512], pv[:])

            psA_cm.__exit__(None, None, None)

            # ---------------- Phase B: attention ----------------
            with (
                tc.tile_pool(name="ex", bufs=3) as expool,
                tc.tile_pool(name="ost", bufs=2) as ostpool,
                tc.tile_pool(name="rcp", bufs=4) as rcpool,
                tc.tile_pool(name="psS", bufs=2, space="PSUM") as psS,
                tc.tile_pool(name="psC", bufs=1, space="PSUM") as psC,
                tc.tile_pool(name="psD", bufs=1, space="PSUM") as psD,
            ):
                def emit_scores(s, t, ps):
                    for o in range(NO):
                        _mm(nc, ps[:],
                            kT16[:, o, t * P:(t + 1) * P],
                            qT16[:, o, s * IB:(s + 1) * IB],
                            start=(o == 0), stop=(o == NO - 1))

                for s in reversed(range(N_IB)):
                    jt_n = JT_SLOTS[s]
                    cps = [
                        [
                            psC.tile([P, 512], F32, tag=f"c{it}{ob}",
                                     name=f"c{s}_{it}{ob}")
                            for ob in range(2)
                        ]
                        for it in range(2)
                    ]
                    dps = [
                        psD.tile([P, 2], F32, tag=f"d{it}", name=f"d{s}_{it}")
                        for it in range(2)
                    ]
                    ps_t = [None] * jt_n
                    ps_t[0] = psS.tile([P, IB], F32, tag="ps", name=f"ps{s}_0")
                    emit_scores(s, 0, ps_t[0])
                    for t in range(jt_n):
                        if t + 1 < jt_n:
                            ps_t[t + 1] = psS.tile([P, IB], F32, tag="ps",
                                                   name=f"ps{s}_{t + 1}")
                            emit_scores(s, t + 1, ps_t[t + 1])
                        ps = ps_t[t]
                        et = expool.tile([P, IB], F16, tag="et", name=f"et{s}_{t}")
                        if t >= jt_n - 4:
                            eraw = expool.tile([P, IB], F16, tag="eraw",
                                               name=f"er{s}_{t}")
                            nc.scalar.activation(
                                eraw[:], ps[:],
                                mybir.ActivationFunctionType.Exp, scale=scale,
                            )
                            col = s * 4 + (t - (jt_n - 4))
                            nc.vector.scalar_tensor_tensor(
                                et[:], t0_t[:], delta_t[:, col:col + 1], eraw[:],
                                op0=mybir.AluOpType.is_le,
                                op1=mybir.AluOpType.mult,
                            )
                        else:
                            nc.scalar.activation(
                                et[:], ps[:],
                                mybir.ActivationFunctionType.Exp, scale=scale,
                            )
                        last = t == jt_n - 1
                        for it in range(2):
                            lhs = et[:, it * P:(it + 1) * P]
                            for ob in range(2):
                                _mm(nc, cps[it][ob][:], lhs,
                                    v_tiles[t][:, ob * 512:(ob + 1) * 512],
                                    start=(t == 0), stop=last)
                            _mm(nc, dps[it][:], lhs, ones_t[:],
                                start=(t == 0), stop=last)
                    for it in range(2):
                        rc = rcpool.tile([P, 1], F32, tag="rc", name=f"rc{s}_{it}")
                        nc.vector.reciprocal(rc[:], dps[it][:, 0:1])
                        ot = ostpool.tile([P, D], F16, tag="ot", name=f"ot{s}_{it}")
                        rows = slice(s * IB + it * P, s * IB + (it + 1) * P)
                        for ob in range(2):
                            cols = slice(ob * 512, (ob + 1) * 512)
                            nc.vector.tensor_scalar_mul(
                                ot[:, cols], cps[it][ob][:], rc[:]
                            )
                            nc.sync.dma_start(out[rows, cols], ot[:, cols])

    nc.compile()
    return nc


_NC_CACHE = None


def _get_nc():
    global _NC_CACHE
    if _NC_CACHE is None:
        _NC_CACHE = build_program()
    return _NC_CACHE


def _pack(a2d):
    """[ND*P, C] d-major -> [P, ND*C] partition-major (SBUF layout)."""
    d, c = a2d.shape
    return np.ascontiguousarray(
        a2d.reshape(ND, P, c).transpose(1, 0, 2).reshape(P, ND * c))


def make_core_inputs(x, Wq, Wk, Wv):
    """Host-side shard prep. Returns list of 8 in_maps."""
    x = np.asarray(x, dtype=np.float32)
    wqT = _pack(np.asarray(Wq, np.float32).T.astype(np.float16))
    wkT = _pack(np.asarray(Wk, np.float32).T.astype(ml_dtypes.float8_e4m3))
    wvT = _pack(np.asarray(Wv, np.float32).T.astype(np.float16))
    t0 = (np.arange(P, dtype=np.float32)[:, None]
          - np.arange(IB, dtype=np.float32)[None, :]).astype(np.float16)
    t0 = np.ascontiguousarray(t0)

    in_maps = []
    for c in range(N_CORES):
        b, r = divmod(c, 2)
        starts = ROLE_STARTS[r]
        xT = np.ascontiguousarray(x[b].T)
        xq = np.concatenate([x[b][i0:i0 + IB, :] for i0 in starts], axis=0)
        xqT = _pack(xq.T.astype(np.float16))
        delta = np.empty((P, 16), np.float16)
        for s in range(N_IB):
            for tr in range(4):
                t = JT_SLOTS[s] - 4 + tr
                delta[:, s * 4 + tr] = float(starts[s] - P * t)
        in_maps.append({
            "xT16": _pack(xT.astype(np.float16)),
            "x8": _pack(xT.astype(ml_dtypes.float8_e4m3)),
            "xqT": xqT,
            "wqT": wqT, "wkT": wkT, "wvT": wvT,
            "t0": t0, "delta": np.ascontiguousarray(delta),
            "ones": np.ones((P, 2), np.float16),
        })
    return in_maps


def assemble_output(results):
    """Gather 8 per-core [1024, 1024] outputs into [B, S, D]."""
    out = np.empty((B, S, D), np.float32)
    for c in range(N_CORES):
        b, r = divmod(c, 2)
        starts = ROLE_STARTS[r]
        oc = results[c]["out"]
        for s, i0 in enumerate(starts):
            out[b, i0:i0 + IB, :] = oc[s * IB:(s + 1) * IB, :].astype(np.float32)
    return out


def kernel(x, Wq, Wk, Wv):
    nc = _get_nc()
    in_maps = make_core_inputs(x, Wq, Wk, Wv)
    res = run_bass_kernel_spmd(nc, in_maps, list(range(N_CORES)))
    return assemble_output(res.results)


# revision 10
# speedup vs baseline: 1.0386x; 1.0094x over previous
"""
Causal self-attention (single head) on 8 trn2 NeuronCores.

Problem: x[4, 2048, 1024], Wq/Wk/Wv[1024, 1024] (torch Linear layout [d_out, d_in]).
    q/k/v = x @ W.T ; out = softmax(mask(q k^T) / 32) @ v

Sharding (no collectives, uniform SPMD program):
  core c -> batch b = c // 2, role r = c % 2.
  Both cores of a pair compute K/V projections for the full 2048-row
  sequence of their batch (duplicated work, avoids cross-core comms).
  Query rows are split between the pair in 4 i-blocks of 256 rows with
  per-slot padded causal extents JT_SLOTS = [4, 8, 12, 16] identical for
  both roles; causality inside the padded slots is enforced with a
  per-core "delta" input (keep iff jj - ii <= delta).

Precision plan (error gate is 2e-2 relative; measured headroom ~10x):
  fp16 is the working dtype (all |values| < 6e4, 10-bit mantissa).
  K-projection runs as fp8e4m3 DoubleRow matmuls (2 weights/PE cell,
  2x MACs/cycle); q/k are stored fp16 and scores run fp16 (storing them
  fp8 + fp8 scores measured 2.4e-2 rel err -- over the gate; fp8 only in
  the K projection measures under it). V path stays fp16 throughout.

Everything is SBUF-resident (~170 KB of the 208 KB/partition): x in
fp16 + fp8, all three weights, kT8/qT8, and the 16 v tiles. No DRAM
spills, so the only DMA is inputs in (~11 MB) and the output out (4 MB).

Phase B is software-pipelined: scores(t+1) is issued to the PE before
ctx(t), so the activation-engine exp(t) latency hides behind the next
scores matmul and the PE never idles (HAM clock gate stays at 8/8).
"""

import sys

for _p in ("/opt/trn_rl_repo", "/root/.axon_site/_ro/trn_rl_repo"):
    if _p not in sys.path:
        sys.path.append(_p)

import numpy as np
import ml_dtypes

import concourse.bass as bass
import concourse.mybir as mybir
import concourse.tile as tile
from concourse import bacc
from concourse.bass_utils import run_bass_kernel_spmd
import concourse.bass_utils as _bu

# NOTE: walrus --enable-ldw-opt stays at its default (false): the measured
# back-to-back MM rate is full speed without it for fp16/fp8 (216.5 ns per
# 512-row MM), and enabling it rejects DoubleRow LDWEIGHTS instructions.

F32 = mybir.dt.float32
F16 = mybir.dt.float16
F8 = mybir.dt.float8e4
DR = mybir.MatmulPerfMode.DoubleRow

B, S, D = 4, 2048, 1024
P = 128
ND = D // P          # 8 d-tiles (projection contraction)
NO = D // P          # 8 o-tiles
IB = 256             # i-block (query block) rows
N_IB = 4
JT_SLOTS = [4, 8, 12, 16]
ROLE_STARTS = {
    0: [0, 768, 1024, 1792],
    1: [256, 512, 1280, 1536],
}
N_CORES = 8
N_WARM = 22


def _mm(nc, out, lhsT, rhs, start, stop, perf_mode=None):
    nc.tensor.matmul(out, lhsT, rhs, start=start, stop=stop,
                     perf_mode=perf_mode)


def build_program():
    nc = bacc.Bacc(
        "TRN2",
        target_bir_lowering=False,
        debug=False,
        enable_asserts=False,
        num_devices=N_CORES,
    )
    # Inputs are host-packed to the SBUF partition-major layout
    # [P, ND*C]: each DMA is 128 contiguous 16KB-ish runs instead of 1024
    # 1KB runs -- descriptor generation (~11ns each) was costing 5-11us of
    # engine issue time per load the d-major way.
    xT16_in = nc.dram_tensor("xT16", [P, ND * S], F16, kind="ExternalInput").ap()
    x8_in = nc.dram_tensor("x8", [P, ND * S], F8, kind="ExternalInput").ap()
    xqT_in = nc.dram_tensor("xqT", [P, ND * N_IB * IB], F16, kind="ExternalInput").ap()
    wq_in = nc.dram_tensor("wqT", [P, ND * D], F16, kind="ExternalInput").ap()
    wk_in = nc.dram_tensor("wkT", [P, ND * D], F8, kind="ExternalInput").ap()
    wv_in = nc.dram_tensor("wvT", [P, ND * D], F16, kind="ExternalInput").ap()
    t0_in = nc.dram_tensor("t0", [P, IB], F16, kind="ExternalInput").ap()
    delta_in = nc.dram_tensor("delta", [P, 16], F16, kind="ExternalInput").ap()
    ones_in = nc.dram_tensor("ones", [P, 2], F16, kind="ExternalInput").ap()
    out = nc.dram_tensor("out", [N_IB * IB, D], F16, kind="ExternalOutput").ap()

    scale = 1.0 / 32.0  # 1/sqrt(d_v)

    def packed(ap2d, c):
        # [P, ND*C] host-packed DRAM view -> [P, ND, C] 3D AP
        return ap2d.rearrange("p (nd c) -> p nd c", c=c)

    with tile.TileContext(nc) as tc:
        with tc.tile_pool(name="res", bufs=1) as rp:
            # ---- constants first (warm-up stationary/moving data) ----
            t0_t = rp.tile([P, IB], F16, tag="t0")
            nc.gpsimd.dma_start(t0_t[:], t0_in[:])
            delta_t = rp.tile([P, 16], F16, tag="delta")
            nc.gpsimd.dma_start(delta_t[:], delta_in[:])
            ones_t = rp.tile([P, 2], F16, tag="ones")
            nc.gpsimd.dma_start(ones_t[:], ones_in[:])

            # ---- resident tensors + their loads ----
            # Two DMA queues, each FIFO-serialized in consumption order so
            # the Q-projection inputs get the full HBM bandwidth first
            # (concurrent queues split it and delayed Q's start by ~11us).
            # First wave is only the 2MB the first Q psum groups need
            # (wq o-half 0 + xq chunk 0), so Q starts ~6us earlier; the
            # rest streams behind it in consumption order, per-queue FIFO.
            xq16 = rp.tile([P, ND, N_IB * IB], F16, tag="xq16")
            xqp = packed(xqT_in, N_IB * IB)
            nc.scalar.dma_start(xq16[:, :, 0:512], xqp[:, :, 0:512])
            wq16 = rp.tile([P, ND, D], F16, tag="wq16")
            wqp = packed(wq_in, D)
            nc.sync.dma_start(wq16[:, :, 0:512], wqp[:, :, 0:512])
            nc.sync.dma_start(wq16[:, :, 512:1024], wqp[:, :, 512:1024])
            nc.scalar.dma_start(xq16[:, :, 512:1024], xqp[:, :, 512:1024])
            wk8 = rp.tile([P, ND, D], F8, tag="wk8")
            nc.sync.dma_start(wk8[:], packed(wk_in, D))
            x8t = rp.tile([P, ND, S], F8, tag="x8t")
            nc.scalar.dma_start(x8t[:], packed(x8_in, S))
            wv16 = rp.tile([P, ND, D], F16, tag="wv16")
            nc.sync.dma_start(wv16[:], packed(wv_in, D))
            xt16 = rp.tile([P, ND, S], F16, tag="xt16")
            nc.scalar.dma_start(xt16[:], packed(xT16_in, S))

            kT16 = rp.tile([P, NO, S], F16, tag="kT16")
            qT16 = rp.tile([P, NO, N_IB * IB], F16, tag="qT16")
            v_tiles = [
                rp.tile([P, D], F16, tag=f"v{j}", name=f"v{j}")
                for j in range(S // P)
            ]

            # ---- PE warm-up on an on-chip memset tile: no DMA dependency,
            # so the HAM ramp starts right after the framework preamble and
            # covers the ~7us DMA cold-start of the first real loads.
            warm16 = rp.tile([P, IB], F16, tag="warm16")
            nc.vector.memset(warm16[:], 1.0)
            psA_cm = tc.tile_pool(name="psA", bufs=2, space="PSUM")
            psA = psA_cm.__enter__()
            wps = psA.tile([P, IB], F32, tag="wps", name="wps", bufs=1)
            for w in range(N_WARM):
                _mm(nc, wps[:], warm16[:, 0:P], warm16[:], start=True, stop=True)

            # ---------------- Phase A: projections ----------------
            # Q: fp16, psum [o 128, i 512]; store as fp8 for the scores mm
            for sb in range(2):
                for o in range(NO):
                    pq = psA.tile([P, 512], F32, tag="pp", name=f"pq{sb}_{o}")
                    for d in range(ND):
                        _mm(nc, pq[:],
                            wq16[:, d, o * P:(o + 1) * P],
                            xq16[:, d, sb * 512:(sb + 1) * 512],
                            start=(d == 0), stop=(d == ND - 1))
                    nc.vector.tensor_copy(
                        qT16[:, o, sb * 512:(sb + 1) * 512], pq[:])

            # K: fp8 DoubleRow, psum [o 128, j 512]; store fp8
            for jb in range(S // 512):
                for o in range(NO):
                    pk = psA.tile([P, 512], F32, tag="pp", name=f"pk{jb}_{o}")
                    for g in range(ND // 2):
                        _mm(nc, pk[:],
                            wk8[:, 2 * g:2 * g + 2, o * P:(o + 1) * P],
                            x8t[:, 2 * g:2 * g + 2, jb * 512:(jb + 1) * 512],
                            start=(g == 0), stop=(g == ND // 2 - 1),
                            perf_mode=DR)
                    nc.vector.tensor_copy(
                        kT16[:, o, jb * 512:(jb + 1) * 512], pk[:])

            # V: fp16, psum [j 128, o 512]; x tile stationary, wv moving
            for jb in range(S // 512):
                for jj in range(4):
                    jt = jb * 4 + jj
                    for ob in range(2):
                        pv = psA.tile([P, 512], F32, tag="pp", name=f"pv{jt}_{ob}")
                        for d in range(ND):
                            _mm(nc, pv[:],
                                xt16[:, d, jt * P:(jt + 1) * P],
                                wv16[:, d, ob * 512:(ob + 1) * 512],
                                start=(d == 0), stop=(d == ND - 1))
                        nc.vector.tensor_copy(
                            v_tiles[jt][:, ob * 512:(ob + 1) * 512], pv[:])

            psA_cm.__exit__(None, None, None)

            # ---------------- Phase B: attention ----------------
            with (
                tc.tile_pool(name="ex", bufs=3) as expool,
                tc.tile_pool(name="ost", bufs=2) as ostpool,
                tc.tile_pool(name="rcp", bufs=4) as rcpool,
                tc.tile_pool(name="psS", bufs=2, space="PSUM") as psS,
                tc.tile_pool(name="psC", bufs=1, space="PSUM") as psC,
                tc.tile_pool(name="psD", bufs=1, space="PSUM") as psD,
            ):
                def emit_scores(s, t, ps):
                    for o in range(NO):
                        _mm(nc, ps[:],
                            kT16[:, o, t * P:(t + 1) * P],
                            qT16[:, o, s * IB:(s + 1) * IB],
                            start=(o == 0), stop=(o == NO - 1))

                for s in reversed(range(N_IB)):
                    jt_n = JT_SLOTS[s]
                    cps = [
                        [
                            psC.tile([P, 512], F32, tag=f"c{it}{ob}",
                                     name=f"c{s}_{it}{ob}")
                            for ob in range(2)
                        ]
                        for it in range(2)
                    ]
                    dps = [
                        psD.tile([P, 2], F32, tag=f"d{it}", name=f"d{s}_{it}")
                        for it in range(2)
                    ]
                    ps_t = [None] * jt_n
                    ps_t[0] = psS.tile([P, IB], F32, tag="ps", name=f"ps{s}_0")
                    emit_scores(s, 0, ps_t[0])
                    for t in range(jt_n):
                        if t + 1 < jt_n:
                            ps_t[t + 1] = psS.tile([P, IB], F32, tag="ps",
                                                   name=f"ps{s}_{t + 1}")
                            emit_scores(s, t + 1, ps_t[t + 1])
                        ps = ps_t[t]
                        et = expool.tile([P, IB], F16, tag="et", name=f"et{s}_{t}")
                        if t >= jt_n - 4:
                            eraw = expool.tile([P, IB], F16, tag="eraw",
                                               name=f"er{s}_{t}")
                            nc.scalar.activation(
                                eraw[:], ps[:],
                                mybir.ActivationFunctionType.Exp, scale=scale,
                            )
                            col = s * 4 + (t - (jt_n - 4))
                            nc.vector.scalar_tensor_tensor(
                                et[:], t0_t[:], delta_t[:, col:col + 1], eraw[:],
                                op0=mybir.AluOpType.is_le,
                                op1=mybir.AluOpType.mult,
                            )
                        else:
                            nc.scalar.activation(
                                et[:], ps[:],
                                mybir.ActivationFunctionType.Exp, scale=scale,
                            )
                        last = t == jt_n - 1
                        for it in range(2):
                            lhs = et[:, it * P:(it + 1) * P]
                            for ob in range(2):
                                _mm(nc, cps[it][ob][:], lhs,
                                    v_tiles[t][:, ob * 512:(ob + 1) * 512],
                                    start=(t == 0), stop=last)
                            _mm(nc, dps[it][:], lhs, ones_t[:],
                                start=(t == 0), stop=last)
                    for it in range(2):
                        rc = rcpool.tile([P, 1], F32, tag="rc", name=f"rc{s}_{it}")
                        nc.vector.reciprocal(rc[:], dps[it][:, 0:1])
                        ot = ostpool.tile([P, D], F16, tag="ot", name=f"ot{s}_{it}")
                        rows = slice(s * IB + it * P, s * IB + (it + 1) * P)
                        for ob in range(2):
                            cols = slice(ob * 512, (ob + 1) * 512)
                            nc.vector.tensor_scalar_mul(
                                ot[:, cols], cps[it][ob][:], rc[:]
                            )
                            nc.sync.dma_start(out[rows, cols], ot[:, cols])

    nc.compile()
    return nc


_NC_CACHE = None


def _get_nc():
    global _NC_CACHE
    if _NC_CACHE is None:
        _NC_CACHE = build_program()
    return _NC_CACHE


def _pack(a2d):
    """[ND*P, C] d-major -> [P, ND*C] partition-major (SBUF layout)."""
    d, c = a2d.shape
    return np.ascontiguousarray(
        a2d.reshape(ND, P, c).transpose(1, 0, 2).reshape(P, ND * c))


def make_core_inputs(x, Wq, Wk, Wv):
    """Host-side shard prep. Returns list of 8 in_maps."""
    x = np.asarray(x, dtype=np.float32)
    wqT = _pack(np.asarray(Wq, np.float32).T.astype(np.float16))
    wkT = _pack(np.asarray(Wk, np.float32).T.astype(ml_dtypes.float8_e4m3))
    wvT = _pack(np.asarray(Wv, np.float32).T.astype(np.float16))
    t0 = (np.arange(P, dtype=np.float32)[:, None]
          - np.arange(IB, dtype=np.float32)[None, :]).astype(np.float16)
    t0 = np.ascontiguousarray(t0)

    in_maps = []
    for c in range(N_CORES):
        b, r = divmod(c, 2)
        starts = ROLE_STARTS[r]
        xT = np.ascontiguousarray(x[b].T)
        xq = np.concatenate([x[b][i0:i0 + IB, :] for i0 in starts], axis=0)
        xqT = _pack(xq.T.astype(np.float16))
        delta = np.empty((P, 16), np.float16)
        for s in range(N_IB):
            for tr in range(4):
                t = JT_SLOTS[s] - 4 + tr
                delta[:, s * 4 + tr] = float(starts[s] - P * t)
        in_maps.append({
            "xT16": _pack(xT.astype(np.float16)),
            "x8": _pack(xT.astype(ml_dtypes.float8_e4m3)),
            "xqT": xqT,
            "wqT": wqT, "wkT": wkT, "wvT": wvT,
            "t0": t0, "delta": np.ascontiguousarray(delta),
            "ones": np.ones((P, 2), np.float16),
        })
    return in_maps


def assemble_output(results):
    """Gather 8 per-core [1024, 1024] outputs into [B, S, D]."""
    out = np.empty((B, S, D), np.float32)
    for c in range(N_CORES):
        b, r = divmod(c, 2)
        starts = ROLE_STARTS[r]
        oc = results[c]["out"]
        for s, i0 in enumerate(starts):
            out[b, i0:i0 + IB, :] = oc[s * IB:(s + 1) * IB, :].astype(np.float32)
    return out


def kernel(x, Wq, Wk, Wv):
    nc = _get_nc()
    in_maps = make_core_inputs(x, Wq, Wk, Wv)
    res = run_bass_kernel_spmd(nc, in_maps, list(range(N_CORES)))
    return assemble_output(res.results)


# revision 11
# speedup vs baseline: 1.0461x; 1.0073x over previous
"""
Causal self-attention (single head) on 8 trn2 NeuronCores.

Problem: x[4, 2048, 1024], Wq/Wk/Wv[1024, 1024] (torch Linear layout [d_out, d_in]).
    q/k/v = x @ W.T ; out = softmax(mask(q k^T) / 32) @ v

Sharding (no collectives, uniform SPMD program):
  core c -> batch b = c // 2, role r = c % 2.
  Both cores of a pair compute K/V projections for the full 2048-row
  sequence of their batch (duplicated work, avoids cross-core comms).
  Query rows are split between the pair in 4 i-blocks of 256 rows with
  per-slot padded causal extents JT_SLOTS = [4, 8, 12, 16] identical for
  both roles; causality inside the padded slots is enforced with a
  per-core "delta" input (keep iff jj - ii <= delta).

Precision plan (error gate is 2e-2 relative; measured headroom ~10x):
  fp16 is the working dtype (all |values| < 6e4, 10-bit mantissa).
  K-projection runs as fp8e4m3 DoubleRow matmuls (2 weights/PE cell,
  2x MACs/cycle); q/k are stored fp16 and scores run fp16 (storing them
  fp8 + fp8 scores measured 2.4e-2 rel err -- over the gate; fp8 only in
  the K projection measures under it). V path stays fp16 throughout.

Everything is SBUF-resident (~170 KB of the 208 KB/partition): x in
fp16 + fp8, all three weights, kT8/qT8, and the 16 v tiles. No DRAM
spills, so the only DMA is inputs in (~11 MB) and the output out (4 MB).

Phase B is software-pipelined: scores(t+1) is issued to the PE before
ctx(t), so the activation-engine exp(t) latency hides behind the next
scores matmul and the PE never idles (HAM clock gate stays at 8/8).
"""

import sys

for _p in ("/opt/trn_rl_repo", "/root/.axon_site/_ro/trn_rl_repo"):
    if _p not in sys.path:
        sys.path.append(_p)

import numpy as np
import ml_dtypes

import concourse.bass as bass
import concourse.mybir as mybir
import concourse.tile as tile
from concourse import bacc
from concourse.bass_utils import run_bass_kernel_spmd
import concourse.bass_utils as _bu

# NOTE: walrus --enable-ldw-opt stays at its default (false): the measured
# back-to-back MM rate is full speed without it for fp16/fp8 (216.5 ns per
# 512-row MM), and enabling it rejects DoubleRow LDWEIGHTS instructions.

F32 = mybir.dt.float32
F16 = mybir.dt.float16
F8 = mybir.dt.float8e4
DR = mybir.MatmulPerfMode.DoubleRow

B, S, D = 4, 2048, 1024
P = 128
ND = D // P          # 8 d-tiles (projection contraction)
NO = D // P          # 8 o-tiles
IB = 256             # i-block (query block) rows
N_IB = 4
JT_SLOTS = [4, 8, 12, 16]
ROLE_STARTS = {
    0: [0, 768, 1024, 1792],
    1: [256, 512, 1280, 1536],
}
N_CORES = 8
N_WARM = 44


def _mm(nc, out, lhsT, rhs, start, stop, perf_mode=None):
    nc.tensor.matmul(out, lhsT, rhs, start=start, stop=stop,
                     perf_mode=perf_mode)


def build_program():
    nc = bacc.Bacc(
        "TRN2",
        target_bir_lowering=False,
        debug=False,
        enable_asserts=False,
        num_devices=N_CORES,
    )
    # Inputs are host-packed to the SBUF partition-major layout
    # [P, ND*C]: each DMA is 128 contiguous 16KB-ish runs instead of 1024
    # 1KB runs -- descriptor generation (~11ns each) was costing 5-11us of
    # engine issue time per load the d-major way.
    xT16_in = nc.dram_tensor("xT16", [P, ND * S], F16, kind="ExternalInput").ap()
    x8_in = nc.dram_tensor("x8", [P, ND * S], F8, kind="ExternalInput").ap()
    xqT_in = nc.dram_tensor("xqT", [P, ND * N_IB * IB], F16, kind="ExternalInput").ap()
    wq_in = nc.dram_tensor("wqT", [P, ND * D], F16, kind="ExternalInput").ap()
    wk_in = nc.dram_tensor("wkT", [P, ND * D], F8, kind="ExternalInput").ap()
    wv_in = nc.dram_tensor("wvT", [P, ND * D], F16, kind="ExternalInput").ap()
    t0_in = nc.dram_tensor("t0", [P, IB], F16, kind="ExternalInput").ap()
    delta_in = nc.dram_tensor("delta", [P, 16], F16, kind="ExternalInput").ap()
    ones_in = nc.dram_tensor("ones", [P, 2], F16, kind="ExternalInput").ap()
    out = nc.dram_tensor("out", [N_IB * IB, D], F16, kind="ExternalOutput").ap()

    scale = 1.0 / 32.0  # 1/sqrt(d_v)

    def packed(ap2d, c):
        # [P, ND*C] host-packed DRAM view -> [P, ND, C] 3D AP
        return ap2d.rearrange("p (nd c) -> p nd c", c=c)

    with tile.TileContext(nc) as tc:
        with tc.tile_pool(name="res", bufs=1) as rp:
            # ---- constants first (warm-up stationary/moving data) ----
            t0_t = rp.tile([P, IB], F16, tag="t0")
            nc.gpsimd.dma_start(t0_t[:], t0_in[:])
            delta_t = rp.tile([P, 16], F16, tag="delta")
            nc.gpsimd.dma_start(delta_t[:], delta_in[:])
            ones_t = rp.tile([P, 2], F16, tag="ones")
            nc.gpsimd.dma_start(ones_t[:], ones_in[:])

            # ---- resident tensors + their loads ----
            # Two DMA queues, each FIFO-serialized in consumption order so
            # the Q-projection inputs get the full HBM bandwidth first
            # (concurrent queues split it and delayed Q's start by ~11us).
            # First wave is only the 2MB the first Q psum groups need
            # (wq o-half 0 + xq chunk 0), so Q starts ~6us earlier; the
            # rest streams behind it in consumption order, per-queue FIFO.
            xq16 = rp.tile([P, ND, N_IB * IB], F16, tag="xq16")
            xqp = packed(xqT_in, N_IB * IB)
            nc.scalar.dma_start(xq16[:, :, 0:512], xqp[:, :, 0:512])
            wq16 = rp.tile([P, ND, D], F16, tag="wq16")
            wqp = packed(wq_in, D)
            nc.sync.dma_start(wq16[:, :, 0:512], wqp[:, :, 0:512])
            nc.sync.dma_start(wq16[:, :, 512:1024], wqp[:, :, 512:1024])
            nc.scalar.dma_start(xq16[:, :, 512:1024], xqp[:, :, 512:1024])
            wk8 = rp.tile([P, ND, D], F8, tag="wk8")
            nc.sync.dma_start(wk8[:], packed(wk_in, D))
            x8t = rp.tile([P, ND, S], F8, tag="x8t")
            nc.scalar.dma_start(x8t[:], packed(x8_in, S))
            wv16 = rp.tile([P, ND, D], F16, tag="wv16")
            nc.sync.dma_start(wv16[:], packed(wv_in, D))
            xt16 = rp.tile([P, ND, S], F16, tag="xt16")
            nc.scalar.dma_start(xt16[:], packed(xT16_in, S))

            kT16 = rp.tile([P, NO, S], F16, tag="kT16")
            qT16 = rp.tile([P, NO, N_IB * IB], F16, tag="qT16")
            v_tiles = [
                rp.tile([P, D], F16, tag=f"v{j}", name=f"v{j}")
                for j in range(S // P)
            ]

            # ---- PE warm-up on an on-chip memset tile: no DMA dependency,
            # so the HAM ramp starts right after the framework preamble and
            # covers the ~7us DMA cold-start of the first real loads.
            warm16 = rp.tile([P, 512], F16, tag="warm16")
            nc.vector.memset(warm16[:], 1.0)
            psA_cm = tc.tile_pool(name="psA", bufs=2, space="PSUM")
            psA = psA_cm.__enter__()
            wps = psA.tile([P, 512], F32, tag="wps", name="wps", bufs=1)
            for w in range(N_WARM):
                _mm(nc, wps[:], warm16[:, 0:P], warm16[:], start=True, stop=True)

            # ---------------- Phase A: projections ----------------
            # Q: fp16, psum [o 128, i 512]; store as fp8 for the scores mm
            for sb in range(2):
                for o in range(NO):
                    pq = psA.tile([P, 512], F32, tag="pp", name=f"pq{sb}_{o}")
                    for d in range(ND):
                        _mm(nc, pq[:],
                            wq16[:, d, o * P:(o + 1) * P],
                            xq16[:, d, sb * 512:(sb + 1) * 512],
                            start=(d == 0), stop=(d == ND - 1))
                    nc.vector.tensor_copy(
                        qT16[:, o, sb * 512:(sb + 1) * 512], pq[:])

            # K: fp8 DoubleRow, psum [o 128, j 512]; store fp8
            for jb in range(S // 512):
                for o in range(NO):
                    pk = psA.tile([P, 512], F32, tag="pp", name=f"pk{jb}_{o}")
                    for g in range(ND // 2):
                        _mm(nc, pk[:],
                            wk8[:, 2 * g:2 * g + 2, o * P:(o + 1) * P],
                            x8t[:, 2 * g:2 * g + 2, jb * 512:(jb + 1) * 512],
                            start=(g == 0), stop=(g == ND // 2 - 1),
                            perf_mode=DR)
                    nc.vector.tensor_copy(
                        kT16[:, o, jb * 512:(jb + 1) * 512], pk[:])

            # V: fp16, psum [j 128, o 512]; x tile stationary, wv moving
            for jb in range(S // 512):
                for jj in range(4):
                    jt = jb * 4 + jj
                    for ob in range(2):
                        pv = psA.tile([P, 512], F32, tag="pp", name=f"pv{jt}_{ob}")
                        for d in range(ND):
                            _mm(nc, pv[:],
                                xt16[:, d, jt * P:(jt + 1) * P],
                                wv16[:, d, ob * 512:(ob + 1) * 512],
                                start=(d == 0), stop=(d == ND - 1))
                        nc.vector.tensor_copy(
                            v_tiles[jt][:, ob * 512:(ob + 1) * 512], pv[:])

            psA_cm.__exit__(None, None, None)

            # ---------------- Phase B: attention ----------------
            with (
                tc.tile_pool(name="ex", bufs=3) as expool,
                tc.tile_pool(name="ost", bufs=2) as ostpool,
                tc.tile_pool(name="rcp", bufs=4) as rcpool,
                tc.tile_pool(name="psS", bufs=2, space="PSUM") as psS,
                tc.tile_pool(name="psC", bufs=1, space="PSUM") as psC,
                tc.tile_pool(name="psD", bufs=1, space="PSUM") as psD,
            ):
                def emit_scores(s, t, ps):
                    for o in range(NO):
                        _mm(nc, ps[:],
                            kT16[:, o, t * P:(t + 1) * P],
                            qT16[:, o, s * IB:(s + 1) * IB],
                            start=(o == 0), stop=(o == NO - 1))

                for s in reversed(range(N_IB)):
                    jt_n = JT_SLOTS[s]
                    cps = [
                        [
                            psC.tile([P, 512], F32, tag=f"c{it}{ob}",
                                     name=f"c{s}_{it}{ob}")
                            for ob in range(2)
                        ]
                        for it in range(2)
                    ]
                    dps = [
                        psD.tile([P, 2], F32, tag=f"d{it}", name=f"d{s}_{it}")
                        for it in range(2)
                    ]
                    ps_t = [None] * jt_n
                    ps_t[0] = psS.tile([P, IB], F32, tag="ps", name=f"ps{s}_0")
                    emit_scores(s, 0, ps_t[0])
                    for t in range(jt_n):
                        if t + 1 < jt_n:
                            ps_t[t + 1] = psS.tile([P, IB], F32, tag="ps",
                                                   name=f"ps{s}_{t + 1}")
                            emit_scores(s, t + 1, ps_t[t + 1])
                        ps = ps_t[t]
                        et = expool.tile([P, IB], F16, tag="et", name=f"et{s}_{t}")
                        if t >= jt_n - 4:
                            eraw = expool.tile([P, IB], F16, tag="eraw",
                                               name=f"er{s}_{t}")
                            nc.scalar.activation(
                                eraw[:], ps[:],
                                mybir.ActivationFunctionType.Exp, scale=scale,
                            )
                            col = s * 4 + (t - (jt_n - 4))
                            nc.vector.scalar_tensor_tensor(
                                et[:], t0_t[:], delta_t[:, col:col + 1], eraw[:],
                                op0=mybir.AluOpType.is_le,
                                op1=mybir.AluOpType.mult,
                            )
                        else:
                            nc.scalar.activation(
                                et[:], ps[:],
                                mybir.ActivationFunctionType.Exp, scale=scale,
                            )
                        last = t == jt_n - 1
                        for it in range(2):
                            lhs = et[:, it * P:(it + 1) * P]
                            for ob in range(2):
                                _mm(nc, cps[it][ob][:], lhs,
                                    v_tiles[t][:, ob * 512:(ob + 1) * 512],
                                    start=(t == 0), stop=last)
                            _mm(nc, dps[it][:], lhs, ones_t[:],
                                start=(t == 0), stop=last)
                    for it in range(2):
                        rc = rcpool.tile([P, 1], F32, tag="rc", name=f"rc{s}_{it}")
                        nc.vector.reciprocal(rc[:], dps[it][:, 0:1])
                        ot = ostpool.tile([P, D], F16, tag="ot", name=f"ot{s}_{it}")
                        rows = slice(s * IB + it * P, s * IB + (it + 1) * P)
                        for ob in range(2):
                            cols = slice(ob * 512, (ob + 1) * 512)
                            nc.vector.tensor_scalar_mul(
                                ot[:, cols], cps[it][ob][:], rc[:]
                            )
                            nc.sync.dma_start(out[rows, cols], ot[:, cols])

    nc.compile()
    return nc


_NC_CACHE = None


def _get_nc():
    global _NC_CACHE
    if _NC_CACHE is None:
        _NC_CACHE = build_program()
    return _NC_CACHE


def _pack(a2d):
    """[ND*P, C] d-major -> [P, ND*C] partition-major (SBUF layout)."""
    d, c = a2d.shape
    return np.ascontiguousarray(
        a2d.reshape(ND, P, c).transpose(1, 0, 2).reshape(P, ND * c))


def make_core_inputs(x, Wq, Wk, Wv):
    """Host-side shard prep. Returns list of 8 in_maps."""
    x = np.asarray(x, dtype=np.float32)
    wqT = _pack(np.asarray(Wq, np.float32).T.astype(np.float16))
    wkT = _pack(np.asarray(Wk, np.float32).T.astype(ml_dtypes.float8_e4m3))
    wvT = _pack(np.asarray(Wv, np.float32).T.astype(np.float16))
    t0 = (np.arange(P, dtype=np.float32)[:, None]
          - np.arange(IB, dtype=np.float32)[None, :]).astype(np.float16)
    t0 = np.ascontiguousarray(t0)

    in_maps = []
    for c in range(N_CORES):
        b, r = divmod(c, 2)
        starts = ROLE_STARTS[r]
        xT = np.ascontiguousarray(x[b].T)
        xq = np.concatenate([x[b][i0:i0 + IB, :] for i0 in starts], axis=0)
        xqT = _pack(xq.T.astype(np.float16))
        delta = np.empty((P, 16), np.float16)
        for s in range(N_IB):
            for tr in range(4):
                t = JT_SLOTS[s] - 4 + tr
                delta[:, s * 4 + tr] = float(starts[s] - P * t)
        in_maps.append({
            "xT16": _pack(xT.astype(np.float16)),
            "x8": _pack(xT.astype(ml_dtypes.float8_e4m3)),
            "xqT": xqT,
            "wqT": wqT, "wkT": wkT, "wvT": wvT,
            "t0": t0, "delta": np.ascontiguousarray(delta),
            "ones": np.ones((P, 2), np.float16),
        })
    return in_maps


def assemble_output(results):
    """Gather 8 per-core [1024, 1024] outputs into [B, S, D]."""
    out = np.empty((B, S, D), np.float32)
    for c in range(N_CORES):
        b, r = divmod(c, 2)
        starts = ROLE_STARTS[r]
        oc = results[c]["out"]
        for s, i0 in enumerate(starts):
            out[b, i0:i0 + IB, :] = oc[s * IB:(s + 1) * IB, :].astype(np.float32)
    return out


def kernel(x, Wq, Wk, Wv):
    nc = _get_nc()
    in_maps = make_core_inputs(x, Wq, Wk, Wv)
    res = run_bass_kernel_spmd(nc, in_maps, list(range(N_CORES)))
    return assemble_output(res.results)


# revision 12
# speedup vs baseline: 1.0547x; 1.0081x over previous
"""
Causal self-attention (single head) on 8 trn2 NeuronCores.

Problem: x[4, 2048, 1024], Wq/Wk/Wv[1024, 1024] (torch Linear layout [d_out, d_in]).
    q/k/v = x @ W.T ; out = softmax(mask(q k^T) / 32) @ v

Sharding (no collectives, uniform SPMD program):
  core c -> batch b = c // 2, role r = c % 2.
  Both cores of a pair compute K/V projections for the full 2048-row
  sequence of their batch (duplicated work, avoids cross-core comms).
  Query rows are split between the pair in 4 i-blocks of 256 rows with
  per-slot padded causal extents JT_SLOTS = [4, 8, 12, 16] identical for
  both roles; causality inside the padded slots is enforced with a
  per-core "delta" input (keep iff jj - ii <= delta).

Precision plan (error gate is 2e-2 relative; measured headroom ~10x):
  fp16 is the working dtype (all |values| < 6e4, 10-bit mantissa).
  K-projection runs as fp8e4m3 DoubleRow matmuls (2 weights/PE cell,
  2x MACs/cycle); q/k are stored fp16 and scores run fp16 (storing them
  fp8 + fp8 scores measured 2.4e-2 rel err -- over the gate; fp8 only in
  the K projection measures under it). V path stays fp16 throughout.

Everything is SBUF-resident (~170 KB of the 208 KB/partition): x in
fp16 + fp8, all three weights, kT8/qT8, and the 16 v tiles. No DRAM
spills, so the only DMA is inputs in (~11 MB) and the output out (4 MB).

Phase B is software-pipelined: scores(t+1) is issued to the PE before
ctx(t), so the activation-engine exp(t) latency hides behind the next
scores matmul and the PE never idles (HAM clock gate stays at 8/8).
"""

import sys

for _p in ("/opt/trn_rl_repo", "/root/.axon_site/_ro/trn_rl_repo"):
    if _p not in sys.path:
        sys.path.append(_p)

import numpy as np
import ml_dtypes

import concourse.bass as bass
import concourse.mybir as mybir
import concourse.tile as tile
from concourse import bacc
from concourse.bass_utils import run_bass_kernel_spmd
import concourse.bass_utils as _bu

# NOTE: walrus --enable-ldw-opt stays at its default (false): the measured
# back-to-back MM rate is full speed without it for fp16/fp8 (216.5 ns per
# 512-row MM), and enabling it rejects DoubleRow LDWEIGHTS instructions.

F32 = mybir.dt.float32
F16 = mybir.dt.float16
F8 = mybir.dt.float8e4
DR = mybir.MatmulPerfMode.DoubleRow

B, S, D = 4, 2048, 1024
P = 128
ND = D // P          # 8 d-tiles (projection contraction)
NO = D // P          # 8 o-tiles
IB = 256             # i-block (query block) rows
N_IB = 4
JT_SLOTS = [4, 8, 12, 16]
ROLE_STARTS = {
    0: [0, 768, 1024, 1792],
    1: [256, 512, 1280, 1536],
}
N_CORES = 8
N_WARM = 24


def _mm(nc, out, lhsT, rhs, start, stop, perf_mode=None):
    nc.tensor.matmul(out, lhsT, rhs, start=start, stop=stop,
                     perf_mode=perf_mode)


def build_program():
    nc = bacc.Bacc(
        "TRN2",
        target_bir_lowering=False,
        debug=False,
        enable_asserts=False,
        num_devices=N_CORES,
    )
    # Inputs are host-packed to the SBUF partition-major layout
    # [P, ND*C]: each DMA is 128 contiguous 16KB-ish runs instead of 1024
    # 1KB runs -- descriptor generation (~11ns each) was costing 5-11us of
    # engine issue time per load the d-major way.
    xT16_in = nc.dram_tensor("xT16", [P, ND * S], F16, kind="ExternalInput").ap()
    x8_in = nc.dram_tensor("x8", [P, ND * S], F8, kind="ExternalInput").ap()
    # wq/xq ship as per-chunk contiguous tensors: the first Q psum groups
    # need only wq half 0 + xq chunk 0 (2MB), and a fully-contiguous
    # [P, 8K] DMA has 128 descriptors vs 1024 for a sliced view.
    xq_ins = [nc.dram_tensor(f"xqT{i}", [P, ND * 512], F16,
                             kind="ExternalInput").ap() for i in range(2)]
    wq_ins = [nc.dram_tensor(f"wqT{i}", [P, ND * 512], F16,
                             kind="ExternalInput").ap() for i in range(2)]
    wk_in = nc.dram_tensor("wkT", [P, ND * D], F8, kind="ExternalInput").ap()
    wv_in = nc.dram_tensor("wvT", [P, ND * D], F16, kind="ExternalInput").ap()
    t0_in = nc.dram_tensor("t0", [P, IB], F16, kind="ExternalInput").ap()
    delta_in = nc.dram_tensor("delta", [P, 16], F16, kind="ExternalInput").ap()
    ones_in = nc.dram_tensor("ones", [P, 2], F16, kind="ExternalInput").ap()
    out = nc.dram_tensor("out", [N_IB * IB, D], F16, kind="ExternalOutput").ap()

    scale = 1.0 / 32.0  # 1/sqrt(d_v)

    def packed(ap2d, c):
        # [P, ND*C] host-packed DRAM view -> [P, ND, C] 3D AP
        return ap2d.rearrange("p (nd c) -> p nd c", c=c)

    with tile.TileContext(nc) as tc:
        with tc.tile_pool(name="res", bufs=1) as rp:
            # ---- constants first (warm-up stationary/moving data) ----
            t0_t = rp.tile([P, IB], F16, tag="t0")
            nc.gpsimd.dma_start(t0_t[:], t0_in[:])
            delta_t = rp.tile([P, 16], F16, tag="delta")
            nc.gpsimd.dma_start(delta_t[:], delta_in[:])
            ones_t = rp.tile([P, 2], F16, tag="ones")
            nc.gpsimd.dma_start(ones_t[:], ones_in[:])

            # ---- resident tensors + their loads ----
            # Two DMA queues, each FIFO-serialized in consumption order so
            # the Q-projection inputs get the full HBM bandwidth first
            # (concurrent queues split it and delayed Q's start by ~11us).
            # First wave is only the 2MB the first Q psum groups need
            # (wq o-half 0 + xq chunk 0), so Q starts ~6us earlier; the
            # rest streams behind it in consumption order, per-queue FIFO.
            xq16 = [rp.tile([P, ND, 512], F16, tag=f"xq16_{i}",
                            name=f"xq16_{i}") for i in range(2)]
            wq16 = [rp.tile([P, ND, 512], F16, tag=f"wq16_{i}",
                            name=f"wq16_{i}") for i in range(2)]
            nc.scalar.dma_start(xq16[0][:], packed(xq_ins[0], 512))
            nc.sync.dma_start(wq16[0][:], packed(wq_ins[0], 512))
            nc.sync.dma_start(wq16[1][:], packed(wq_ins[1], 512))
            nc.scalar.dma_start(xq16[1][:], packed(xq_ins[1], 512))
            wk8 = rp.tile([P, ND, D], F8, tag="wk8")
            nc.sync.dma_start(wk8[:], packed(wk_in, D))
            x8t = rp.tile([P, ND, S], F8, tag="x8t")
            nc.scalar.dma_start(x8t[:], packed(x8_in, S))
            wv16 = rp.tile([P, ND, D], F16, tag="wv16")
            nc.sync.dma_start(wv16[:], packed(wv_in, D))
            xt16 = rp.tile([P, ND, S], F16, tag="xt16")
            nc.scalar.dma_start(xt16[:], packed(xT16_in, S))

            kT16 = rp.tile([P, NO, S], F16, tag="kT16")
            qT16 = rp.tile([P, NO, N_IB * IB], F16, tag="qT16")
            v_tiles = [
                rp.tile([P, D], F16, tag=f"v{j}", name=f"v{j}")
                for j in range(S // P)
            ]

            # ---- PE warm-up on an on-chip memset tile: no DMA dependency,
            # so the HAM ramp starts right after the framework preamble and
            # covers the ~7us DMA cold-start of the first real loads.
            warm16 = rp.tile([P, 512], F16, tag="warm16")
            nc.vector.memset(warm16[:], 1.0)
            psA_cm = tc.tile_pool(name="psA", bufs=2, space="PSUM")
            psA = psA_cm.__enter__()
            wps = psA.tile([P, 512], F32, tag="wps", name="wps", bufs=1)
            for w in range(N_WARM):
                _mm(nc, wps[:], warm16[:, 0:P], warm16[:], start=True, stop=True)

            # ---------------- Phase A: projections ----------------
            # Q: fp16, psum [o 128, i 512]; store as fp8 for the scores mm
            for sb in range(2):
                for o in range(NO):
                    pq = psA.tile([P, 512], F32, tag="pp", name=f"pq{sb}_{o}")
                    for d in range(ND):
                        _mm(nc, pq[:],
                            wq16[o // 4][:, d, (o % 4) * P:(o % 4 + 1) * P],
                            xq16[sb][:, d, :],
                            start=(d == 0), stop=(d == ND - 1))
                    nc.vector.tensor_copy(
                        qT16[:, o, sb * 512:(sb + 1) * 512], pq[:])

            # K: fp8 DoubleRow, psum [o 128, j 512]; store fp8
            for jb in range(S // 512):
                for o in range(NO):
                    pk = psA.tile([P, 512], F32, tag="pp", name=f"pk{jb}_{o}")
                    for g in range(ND // 2):
                        _mm(nc, pk[:],
                            wk8[:, 2 * g:2 * g + 2, o * P:(o + 1) * P],
                            x8t[:, 2 * g:2 * g + 2, jb * 512:(jb + 1) * 512],
                            start=(g == 0), stop=(g == ND // 2 - 1),
                            perf_mode=DR)
                    nc.vector.tensor_copy(
                        kT16[:, o, jb * 512:(jb + 1) * 512], pk[:])

            # V: fp16, psum [j 128, o 512]; x tile stationary, wv moving
            for jb in range(S // 512):
                for jj in range(4):
                    jt = jb * 4 + jj
                    for ob in range(2):
                        pv = psA.tile([P, 512], F32, tag="pp", name=f"pv{jt}_{ob}")
                        for d in range(ND):
                            _mm(nc, pv[:],
                                xt16[:, d, jt * P:(jt + 1) * P],
                                wv16[:, d, ob * 512:(ob + 1) * 512],
                                start=(d == 0), stop=(d == ND - 1))
                        nc.vector.tensor_copy(
                            v_tiles[jt][:, ob * 512:(ob + 1) * 512], pv[:])

            psA_cm.__exit__(None, None, None)

            # ---------------- Phase B: attention ----------------
            with (
                tc.tile_pool(name="ex", bufs=3) as expool,
                tc.tile_pool(name="ost", bufs=2) as ostpool,
                tc.tile_pool(name="rcp", bufs=4) as rcpool,
                tc.tile_pool(name="psS", bufs=2, space="PSUM") as psS,
                tc.tile_pool(name="psC", bufs=1, space="PSUM") as psC,
                tc.tile_pool(name="psD", bufs=1, space="PSUM") as psD,
            ):
                def emit_scores(s, t, ps):
                    for o in range(NO):
                        _mm(nc, ps[:],
                            kT16[:, o, t * P:(t + 1) * P],
                            qT16[:, o, s * IB:(s + 1) * IB],
                            start=(o == 0), stop=(o == NO - 1))

                for s in reversed(range(N_IB)):
                    jt_n = JT_SLOTS[s]
                    cps = [
                        [
                            psC.tile([P, 512], F32, tag=f"c{it}{ob}",
                                     name=f"c{s}_{it}{ob}")
                            for ob in range(2)
                        ]
                        for it in range(2)
                    ]
                    dps = [
                        psD.tile([P, 2], F32, tag=f"d{it}", name=f"d{s}_{it}")
                        for it in range(2)
                    ]
                    ps_t = [None] * jt_n
                    ps_t[0] = psS.tile([P, IB], F32, tag="ps", name=f"ps{s}_0")
                    emit_scores(s, 0, ps_t[0])
                    for t in range(jt_n):
                        if t + 1 < jt_n:
                            ps_t[t + 1] = psS.tile([P, IB], F32, tag="ps",
                                                   name=f"ps{s}_{t + 1}")
                            emit_scores(s, t + 1, ps_t[t + 1])
                        ps = ps_t[t]
                        et = expool.tile([P, IB], F16, tag="et", name=f"et{s}_{t}")
                        if t >= jt_n - 4:
                            eraw = expool.tile([P, IB], F16, tag="eraw",
                                               name=f"er{s}_{t}")
                            nc.scalar.activation(
                                eraw[:], ps[:],
                                mybir.ActivationFunctionType.Exp, scale=scale,
                            )
                            col = s * 4 + (t - (jt_n - 4))
                            nc.vector.scalar_tensor_tensor(
                                et[:], t0_t[:], delta_t[:, col:col + 1], eraw[:],
                                op0=mybir.AluOpType.is_le,
                                op1=mybir.AluOpType.mult,
                            )
                        else:
                            nc.scalar.activation(
                                et[:], ps[:],
                                mybir.ActivationFunctionType.Exp, scale=scale,
                            )
                        last = t == jt_n - 1
                        for it in range(2):
                            lhs = et[:, it * P:(it + 1) * P]
                            for ob in range(2):
                                _mm(nc, cps[it][ob][:], lhs,
                                    v_tiles[t][:, ob * 512:(ob + 1) * 512],
                                    start=(t == 0), stop=last)
                            _mm(nc, dps[it][:], lhs, ones_t[:],
                                start=(t == 0), stop=last)
                    for it in range(2):
                        rc = rcpool.tile([P, 1], F32, tag="rc", name=f"rc{s}_{it}")
                        nc.vector.reciprocal(rc[:], dps[it][:, 0:1])
                        ot = ostpool.tile([P, D], F16, tag="ot", name=f"ot{s}_{it}")
                        rows = slice(s * IB + it * P, s * IB + (it + 1) * P)
                        for ob in range(2):
                            cols = slice(ob * 512, (ob + 1) * 512)
                            nc.vector.tensor_scalar_mul(
                                ot[:, cols], cps[it][ob][:], rc[:]
                            )
                            nc.sync.dma_start(out[rows, cols], ot[:, cols])

    nc.compile()
    return nc


_NC_CACHE = None


def _get_nc():
    global _NC_CACHE
    if _NC_CACHE is None:
        _NC_CACHE = build_program()
    return _NC_CACHE


def _pack(a2d):
    """[ND*P, C] d-major -> [P, ND*C] partition-major (SBUF layout)."""
    d, c = a2d.shape
    return np.ascontiguousarray(
        a2d.reshape(ND, P, c).transpose(1, 0, 2).reshape(P, ND * c))


def make_core_inputs(x, Wq, Wk, Wv):
    """Host-side shard prep. Returns list of 8 in_maps."""
    x = np.asarray(x, dtype=np.float32)
    wqT_f = np.asarray(Wq, np.float32).T.astype(np.float16)
    wq0 = _pack(wqT_f[:, 0:512])
    wq1 = _pack(wqT_f[:, 512:1024])
    wkT = _pack(np.asarray(Wk, np.float32).T.astype(ml_dtypes.float8_e4m3))
    wvT = _pack(np.asarray(Wv, np.float32).T.astype(np.float16))
    t0 = (np.arange(P, dtype=np.float32)[:, None]
          - np.arange(IB, dtype=np.float32)[None, :]).astype(np.float16)
    t0 = np.ascontiguousarray(t0)

    in_maps = []
    for c in range(N_CORES):
        b, r = divmod(c, 2)
        starts = ROLE_STARTS[r]
        xT = np.ascontiguousarray(x[b].T)
        xq = np.concatenate([x[b][i0:i0 + IB, :] for i0 in starts], axis=0)
        xqT_f = xq.T.astype(np.float16)
        xq0 = _pack(xqT_f[:, 0:512])
        xq1 = _pack(xqT_f[:, 512:1024])
        delta = np.empty((P, 16), np.float16)
        for s in range(N_IB):
            for tr in range(4):
                t = JT_SLOTS[s] - 4 + tr
                delta[:, s * 4 + tr] = float(starts[s] - P * t)
        in_maps.append({
            "xT16": _pack(xT.astype(np.float16)),
            "x8": _pack(xT.astype(ml_dtypes.float8_e4m3)),
            "xqT0": xq0, "xqT1": xq1,
            "wqT0": wq0, "wqT1": wq1,
            "wkT": wkT, "wvT": wvT,
            "t0": t0, "delta": np.ascontiguousarray(delta),
            "ones": np.ones((P, 2), np.float16),
        })
    return in_maps


def assemble_output(results):
    """Gather 8 per-core [1024, 1024] outputs into [B, S, D]."""
    out = np.empty((B, S, D), np.float32)
    for c in range(N_CORES):
        b, r = divmod(c, 2)
        starts = ROLE_STARTS[r]
        oc = results[c]["out"]
        for s, i0 in enumerate(starts):
            out[b, i0:i0 + IB, :] = oc[s * IB:(s + 1) * IB, :].astype(np.float32)
    return out


def kernel(x, Wq, Wk, Wv):
    nc = _get_nc()
    in_maps = make_core_inputs(x, Wq, Wk, Wv)
    res = run_bass_kernel_spmd(nc, in_maps, list(range(N_CORES)))
    return assemble_output(res.results)


# revision 13
# speedup vs baseline: 1.0550x; 1.0003x over previous
"""
Causal self-attention (single head) on 8 trn2 NeuronCores.

Problem: x[4, 2048, 1024], Wq/Wk/Wv[1024, 1024] (torch Linear layout [d_out, d_in]).
    q/k/v = x @ W.T ; out = softmax(mask(q k^T) / 32) @ v

Sharding (no collectives, uniform SPMD program):
  core c -> batch b = c // 2, role r = c % 2.
  Both cores of a pair compute K/V projections for the full 2048-row
  sequence of their batch (duplicated work, avoids cross-core comms).
  Query rows are split between the pair in 4 i-blocks of 256 rows with
  per-slot padded causal extents JT_SLOTS = [4, 8, 12, 16] identical for
  both roles; causality inside the padded slots is enforced with a
  per-core "delta" input (keep iff jj - ii <= delta).

Precision plan (error gate is 2e-2 relative; measured 1.45e-2):
  fp16 is the working dtype (all |values| well inside fp16 range).
  K-projection runs as fp8e4m3 DoubleRow matmuls (2 weights/PE cell,
  2x MACs/cycle, measured 222ns per 256-deep x 512-wide MM vs 2x217ns
  in fp16); q/k are stored fp16 and scores run fp16 (fp8 q/k stores +
  fp8 scores measured 2.4e-2 -- over the gate). V path stays fp16.
  The output DMAs as fp16 and is widened to fp32 on the host.

Everything is SBUF-resident (~170 KB of the 208 KB/partition): x in
fp16 + fp8, all three weights, kT16/qT16, and the 16 v tiles. No DRAM
spills; the only DMA is inputs in (~11 MB) and the output (2 MB).
Inputs are host-packed to the SBUF partition-major layout so each load
is 128 contiguous runs (sliced d-major views cost ~11ns/descriptor x
1024 descriptors of engine issue time per load, and the transfers ran
at half bandwidth). The two tensors the first Q psum groups read ship
as their own contiguous tensors and load first; PE warm-up matmuls on
a memset tile bridge the DMA cold start (~6.5us framework preamble +
~11us first-wave DMA) while holding the HAM clock gate at 8/8.

Phase B is software-pipelined: scores(t+1) is issued to the PE before
ctx(t), so the activation-engine exp(t) latency hides behind the next
scores matmul and the PE never idles. Measured PE idle inside the
kernel body: ~1-4us total.
"""

import sys

for _p in ("/opt/trn_rl_repo", "/root/.axon_site/_ro/trn_rl_repo"):
    if _p not in sys.path:
        sys.path.append(_p)

import numpy as np
import ml_dtypes

import concourse.bass as bass
import concourse.mybir as mybir
import concourse.tile as tile
from concourse import bacc
from concourse.bass_utils import run_bass_kernel_spmd
import concourse.bass_utils as _bu

# NOTE: walrus --enable-ldw-opt stays at its default (false): the measured
# back-to-back MM rate is full speed without it for fp16/fp8 (216.5 ns per
# 512-row MM), and enabling it rejects DoubleRow LDWEIGHTS instructions.

F32 = mybir.dt.float32
F16 = mybir.dt.float16
F8 = mybir.dt.float8e4
DR = mybir.MatmulPerfMode.DoubleRow

B, S, D = 4, 2048, 1024
P = 128
ND = D // P          # 8 d-tiles (projection contraction)
NO = D // P          # 8 o-tiles
IB = 256             # i-block (query block) rows
N_IB = 4
JT_SLOTS = [4, 8, 12, 16]
ROLE_STARTS = {
    0: [0, 768, 1024, 1792],
    1: [256, 512, 1280, 1536],
}
N_CORES = 8
N_WARM = 38


def _mm(nc, out, lhsT, rhs, start, stop, perf_mode=None):
    nc.tensor.matmul(out, lhsT, rhs, start=start, stop=stop,
                     perf_mode=perf_mode)


def build_program():
    nc = bacc.Bacc(
        "TRN2",
        target_bir_lowering=False,
        debug=False,
        enable_asserts=False,
        num_devices=N_CORES,
    )
    # Inputs are host-packed to the SBUF partition-major layout
    # [P, ND*C]: each DMA is 128 contiguous 16KB-ish runs instead of 1024
    # 1KB runs -- descriptor generation (~11ns each) was costing 5-11us of
    # engine issue time per load the d-major way.
    xT16_in = nc.dram_tensor("xT16", [P, ND * S], F16, kind="ExternalInput").ap()
    x8_in = nc.dram_tensor("x8", [P, ND * S], F8, kind="ExternalInput").ap()
    # wq/xq ship as per-chunk contiguous tensors: the first Q psum groups
    # need only wq half 0 + xq chunk 0 (2MB), and a fully-contiguous
    # [P, 8K] DMA has 128 descriptors vs 1024 for a sliced view.
    xq_ins = [nc.dram_tensor(f"xqT{i}", [P, ND * 512], F16,
                             kind="ExternalInput").ap() for i in range(2)]
    wq_ins = [nc.dram_tensor(f"wqT{i}", [P, ND * 512], F16,
                             kind="ExternalInput").ap() for i in range(2)]
    wk_in = nc.dram_tensor("wkT", [P, ND * D], F8, kind="ExternalInput").ap()
    wv_in = nc.dram_tensor("wvT", [P, ND * D], F16, kind="ExternalInput").ap()
    t0_in = nc.dram_tensor("t0", [P, IB], F16, kind="ExternalInput").ap()
    delta_in = nc.dram_tensor("delta", [P, 16], F16, kind="ExternalInput").ap()
    ones_in = nc.dram_tensor("ones", [P, 2], F16, kind="ExternalInput").ap()
    out = nc.dram_tensor("out", [N_IB * IB, D], F16, kind="ExternalOutput").ap()

    scale = 1.0 / 32.0  # 1/sqrt(d_v)

    def packed(ap2d, c):
        # [P, ND*C] host-packed DRAM view -> [P, ND, C] 3D AP
        return ap2d.rearrange("p (nd c) -> p nd c", c=c)

    with tile.TileContext(nc) as tc:
        with tc.tile_pool(name="res", bufs=1) as rp:
            # ---- constants first (warm-up stationary/moving data) ----
            t0_t = rp.tile([P, IB], F16, tag="t0")
            nc.gpsimd.dma_start(t0_t[:], t0_in[:])
            delta_t = rp.tile([P, 16], F16, tag="delta")
            nc.gpsimd.dma_start(delta_t[:], delta_in[:])
            ones_t = rp.tile([P, 2], F16, tag="ones")
            nc.gpsimd.dma_start(ones_t[:], ones_in[:])

            # ---- resident tensors + their loads ----
            # Two DMA queues, each FIFO-serialized in consumption order so
            # the Q-projection inputs get the full HBM bandwidth first
            # (concurrent queues split it and delayed Q's start by ~11us).
            # First wave is only the 2MB the first Q psum groups need
            # (wq o-half 0 + xq chunk 0), so Q starts ~6us earlier; the
            # rest streams behind it in consumption order, per-queue FIFO.
            xq16 = [rp.tile([P, ND, 512], F16, tag=f"xq16_{i}",
                            name=f"xq16_{i}") for i in range(2)]
            wq16 = [rp.tile([P, ND, 512], F16, tag=f"wq16_{i}",
                            name=f"wq16_{i}") for i in range(2)]
            nc.scalar.dma_start(xq16[0][:], packed(xq_ins[0], 512))
            nc.sync.dma_start(wq16[0][:], packed(wq_ins[0], 512))
            nc.sync.dma_start(wq16[1][:], packed(wq_ins[1], 512))
            nc.scalar.dma_start(xq16[1][:], packed(xq_ins[1], 512))
            wk8 = rp.tile([P, ND, D], F8, tag="wk8")
            nc.sync.dma_start(wk8[:], packed(wk_in, D))
            x8t = rp.tile([P, ND, S], F8, tag="x8t")
            nc.scalar.dma_start(x8t[:], packed(x8_in, S))
            wv16 = rp.tile([P, ND, D], F16, tag="wv16")
            nc.sync.dma_start(wv16[:], packed(wv_in, D))
            xt16 = rp.tile([P, ND, S], F16, tag="xt16")
            nc.scalar.dma_start(xt16[:], packed(xT16_in, S))

            kT16 = rp.tile([P, NO, S], F16, tag="kT16")
            qT16 = rp.tile([P, NO, N_IB * IB], F16, tag="qT16")
            v_tiles = [
                rp.tile([P, D], F16, tag=f"v{j}", name=f"v{j}")
                for j in range(S // P)
            ]

            # ---- PE warm-up on an on-chip memset tile: no DMA dependency,
            # so the HAM ramp starts right after the framework preamble and
            # covers the ~7us DMA cold-start of the first real loads.
            warm16 = rp.tile([P, 512], F16, tag="warm16")
            nc.vector.memset(warm16[:], 1.0)
            psA_cm = tc.tile_pool(name="psA", bufs=2, space="PSUM")
            psA = psA_cm.__enter__()
            wps = psA.tile([P, 512], F32, tag="wps", name="wps", bufs=1)
            for w in range(N_WARM):
                _mm(nc, wps[:], warm16[:, 0:P], warm16[:], start=True, stop=True)

            # ---------------- Phase A: projections ----------------
            # Q: fp16, psum [o 128, i 512]; store as fp8 for the scores mm
            for sb in range(2):
                for o in range(NO):
                    pq = psA.tile([P, 512], F32, tag="pp", name=f"pq{sb}_{o}")
                    for d in range(ND):
                        _mm(nc, pq[:],
                            wq16[o // 4][:, d, (o % 4) * P:(o % 4 + 1) * P],
                            xq16[sb][:, d, :],
                            start=(d == 0), stop=(d == ND - 1))
                    nc.vector.tensor_copy(
                        qT16[:, o, sb * 512:(sb + 1) * 512], pq[:])

            # K: fp8 DoubleRow, psum [o 128, j 512]; store fp8
            for jb in range(S // 512):
                for o in range(NO):
                    pk = psA.tile([P, 512], F32, tag="pp", name=f"pk{jb}_{o}")
                    for g in range(ND // 2):
                        _mm(nc, pk[:],
                            wk8[:, 2 * g:2 * g + 2, o * P:(o + 1) * P],
                            x8t[:, 2 * g:2 * g + 2, jb * 512:(jb + 1) * 512],
                            start=(g == 0), stop=(g == ND // 2 - 1),
                            perf_mode=DR)
                    nc.vector.tensor_copy(
                        kT16[:, o, jb * 512:(jb + 1) * 512], pk[:])

            # V: fp16, psum [j 128, o 512]; x tile stationary, wv moving
            for jb in range(S // 512):
                for jj in range(4):
                    jt = jb * 4 + jj
                    for ob in range(2):
                        pv = psA.tile([P, 512], F32, tag="pp", name=f"pv{jt}_{ob}")
                        for d in range(ND):
                            _mm(nc, pv[:],
                                xt16[:, d, jt * P:(jt + 1) * P],
                                wv16[:, d, ob * 512:(ob + 1) * 512],
                                start=(d == 0), stop=(d == ND - 1))
                        nc.vector.tensor_copy(
                            v_tiles[jt][:, ob * 512:(ob + 1) * 512], pv[:])

            psA_cm.__exit__(None, None, None)

            # ---------------- Phase B: attention ----------------
            with (
                tc.tile_pool(name="ex", bufs=3) as expool,
                tc.tile_pool(name="ost", bufs=2) as ostpool,
                tc.tile_pool(name="rcp", bufs=4) as rcpool,
                tc.tile_pool(name="psS", bufs=2, space="PSUM") as psS,
                tc.tile_pool(name="psC", bufs=1, space="PSUM") as psC,
                tc.tile_pool(name="psD", bufs=1, space="PSUM") as psD,
            ):
                def emit_scores(s, t, ps):
                    for o in range(NO):
                        _mm(nc, ps[:],
                            kT16[:, o, t * P:(t + 1) * P],
                            qT16[:, o, s * IB:(s + 1) * IB],
                            start=(o == 0), stop=(o == NO - 1))

                for s in reversed(range(N_IB)):
                    jt_n = JT_SLOTS[s]
                    cps = [
                        [
                            psC.tile([P, 512], F32, tag=f"c{it}{ob}",
                                     name=f"c{s}_{it}{ob}")
                            for ob in range(2)
                        ]
                        for it in range(2)
                    ]
                    dps = [
                        psD.tile([P, 2], F32, tag=f"d{it}", name=f"d{s}_{it}")
                        for it in range(2)
                    ]
                    ps_t = [None] * jt_n
                    ps_t[0] = psS.tile([P, IB], F32, tag="ps", name=f"ps{s}_0")
                    emit_scores(s, 0, ps_t[0])
                    for t in range(jt_n):
                        if t + 1 < jt_n:
                            ps_t[t + 1] = psS.tile([P, IB], F32, tag="ps",
                                                   name=f"ps{s}_{t + 1}")
                            emit_scores(s, t + 1, ps_t[t + 1])
                        ps = ps_t[t]
                        et = expool.tile([P, IB], F16, tag="et", name=f"et{s}_{t}")
                        if t >= jt_n - 4:
                            eraw = expool.tile([P, IB], F16, tag="eraw",
                                               name=f"er{s}_{t}")
                            nc.scalar.activation(
                                eraw[:], ps[:],
                                mybir.ActivationFunctionType.Exp, scale=scale,
                            )
                            col = s * 4 + (t - (jt_n - 4))
                            nc.vector.scalar_tensor_tensor(
                                et[:], t0_t[:], delta_t[:, col:col + 1], eraw[:],
                                op0=mybir.AluOpType.is_le,
                                op1=mybir.AluOpType.mult,
                            )
                        else:
                            nc.scalar.activation(
                                et[:], ps[:],
                                mybir.ActivationFunctionType.Exp, scale=scale,
                            )
                        last = t == jt_n - 1
                        for it in range(2):
                            lhs = et[:, it * P:(it + 1) * P]
                            for ob in range(2):
                                _mm(nc, cps[it][ob][:], lhs,
                                    v_tiles[t][:, ob * 512:(ob + 1) * 512],
                                    start=(t == 0), stop=last)
                            _mm(nc, dps[it][:], lhs, ones_t[:],
                                start=(t == 0), stop=last)
                    for it in range(2):
                        rc = rcpool.tile([P, 1], F32, tag="rc", name=f"rc{s}_{it}")
                        nc.vector.reciprocal(rc[:], dps[it][:, 0:1])
                        ot = ostpool.tile([P, D], F16, tag="ot", name=f"ot{s}_{it}")
                        rows = slice(s * IB + it * P, s * IB + (it + 1) * P)
                        for ob in range(2):
                            cols = slice(ob * 512, (ob + 1) * 512)
                            nc.vector.tensor_scalar_mul(
                                ot[:, cols], cps[it][ob][:], rc[:]
                            )
                            nc.sync.dma_start(out[rows, cols], ot[:, cols])

    nc.compile()
    return nc


_NC_CACHE = None


def _get_nc():
    global _NC_CACHE
    if _NC_CACHE is None:
        _NC_CACHE = build_program()
    return _NC_CACHE


def _pack(a2d):
    """[ND*P, C] d-major -> [P, ND*C] partition-major (SBUF layout)."""
    d, c = a2d.shape
    return np.ascontiguousarray(
        a2d.reshape(ND, P, c).transpose(1, 0, 2).reshape(P, ND * c))


def make_core_inputs(x, Wq, Wk, Wv):
    """Host-side shard prep. Returns list of 8 in_maps."""
    x = np.asarray(x, dtype=np.float32)
    wqT_f = np.asarray(Wq, np.float32).T.astype(np.float16)
    wq0 = _pack(wqT_f[:, 0:512])
    wq1 = _pack(wqT_f[:, 512:1024])
    wkT = _pack(np.asarray(Wk, np.float32).T.astype(ml_dtypes.float8_e4m3))
    wvT = _pack(np.asarray(Wv, np.float32).T.astype(np.float16))
    t0 = (np.arange(P, dtype=np.float32)[:, None]
          - np.arange(IB, dtype=np.float32)[None, :]).astype(np.float16)
    t0 = np.ascontiguousarray(t0)

    in_maps = []
    for c in range(N_CORES):
        b, r = divmod(c, 2)
        starts = ROLE_STARTS[r]
        xT = np.ascontiguousarray(x[b].T)
        xq = np.concatenate([x[b][i0:i0 + IB, :] for i0 in starts], axis=0)
        xqT_f = xq.T.astype(np.float16)
        xq0 = _pack(xqT_f[:, 0:512])
        xq1 = _pack(xqT_f[:, 512:1024])
        delta = np.empty((P, 16), np.float16)
        for s in range(N_IB):
            for tr in range(4):
                t = JT_SLOTS[s] - 4 + tr
                delta[:, s * 4 + tr] = float(starts[s] - P * t)
        in_maps.append({
            "xT16": _pack(xT.astype(np.float16)),
            "x8": _pack(xT.astype(ml_dtypes.float8_e4m3)),
            "xqT0": xq0, "xqT1": xq1,
            "wqT0": wq0, "wqT1": wq1,
            "wkT": wkT, "wvT": wvT,
            "t0": t0, "delta": np.ascontiguousarray(delta),
            "ones": np.ones((P, 2), np.float16),
        })
    return in_maps


def assemble_output(results):
    """Gather 8 per-core [1024, 1024] outputs into [B, S, D]."""
    out = np.empty((B, S, D), np.float32)
    for c in range(N_CORES):
        b, r = divmod(c, 2)
        starts = ROLE_STARTS[r]
        oc = results[c]["out"]
        for s, i0 in enumerate(starts):
            out[b, i0:i0 + IB, :] = oc[s * IB:(s + 1) * IB, :].astype(np.float32)
    return out


def kernel(x, Wq, Wk, Wv):
    nc = _get_nc()
    in_maps = make_core_inputs(x, Wq, Wk, Wv)
    res = run_bass_kernel_spmd(nc, in_maps, list(range(N_CORES)))
    return assemble_output(res.results)
